# revision 1
# baseline (speedup 1.0000x reference)
"""Trainium2 Bass/Tile kernel for nn_EncoderLayer (dense transformer block).

Strategy: pure data-parallel over batch (B=8 -> 1 batch element per core,
no collectives). Per core, activations are kept feature-major ([D, T]) so
every matmul contracts over the partition axis with weights loaded in their
natural [D_in, D_out] layout; attention P@V consumes row-major V with an
appended mask column that yields the softmax normalizer for free; the FFN2
output is produced row-major (via ffT as the stationary operand) so the
final LayerNorm reduces along the free axis and the result DMAs out
contiguously. Residuals are folded into matmul accumulation chains with an
identity stationary/moving operand. All matmuls run as float32r (full-rate
fp32 PE mode).
"""

import json
import sys

if "/opt/trn_rl_repo" not in sys.path:
    sys.path.insert(0, "/opt/trn_rl_repo")

import numpy as np

import concourse.bass as bass
import concourse.mybir as mybir
import concourse.tile as tile

B, T, CC, DM, H, DH, DFF, K = 8, 1024, 256, 1024, 16, 64, 5120, 3
EMB = CC + DM  # 1280
EPS = 1e-6
f32 = mybir.dt.float32
f32r = mybir.dt.float32r
AF = mybir.ActivationFunctionType
OP = mybir.AluOpType

NT = T // 128          # 8 time tiles
NKE = EMB // 128       # 10 embed k-tiles
NKD = DM // 128        # 8 d_model k-tiles
NMF = DFF // 128       # 40 d_ff tiles
HV = DH + 1            # 65: per-head V columns + mask column


def _mm(nc, out, lhsT, rhs, start, stop):
    nc.tensor.matmul(out, lhsT.bitcast(f32r), rhs.bitcast(f32r), start=start, stop=stop)


def build_nc(phase=99):
    import os
    phase = int(os.environ.get("KPHASE", phase))
    nc = bass.Bass()

    xt_d = nc.declare_dram_parameter("xt", [EMB, T], f32, isOutput=False)
    maskf_d = nc.declare_dram_parameter("maskf", [T], f32, isOutput=False)
    seqf_d = nc.declare_dram_parameter("seqf", [T], f32, isOutput=False)
    convw_d = nc.declare_dram_parameter("convw", [K], f32, isOutput=False)
    wqr_d = nc.declare_dram_parameter("wqr", [8, 8, 128, 128], f32, isOutput=False)
    wkr_d = nc.declare_dram_parameter("wkr", [8, 8, 128, 128], f32, isOutput=False)
    wv_d = nc.declare_dram_parameter("wv", [DM, DM], f32, isOutput=False)
    wor_d = nc.declare_dram_parameter("wor", [8, 8, 128, 128], f32, isOutput=False)
    w1r_d = nc.declare_dram_parameter("w1r", [40, 10, 128, 128], f32, isOutput=False)
    w2_d = nc.declare_dram_parameter("w2", [DFF, EMB], f32, isOutput=False)
    bq_d = nc.declare_dram_parameter("bq", [DM], f32, isOutput=False)
    bk_d = nc.declare_dram_parameter("bk", [DM], f32, isOutput=False)
    bv_d = nc.declare_dram_parameter("bv", [DM], f32, isOutput=False)
    bo_d = nc.declare_dram_parameter("bo", [DM], f32, isOutput=False)
    b1_d = nc.declare_dram_parameter("b1", [DFF], f32, isOutput=False)
    b2_d = nc.declare_dram_parameter("b2", [EMB], f32, isOutput=False)
    g1_d = nc.declare_dram_parameter("g1", [EMB], f32, isOutput=False)
    beta1_d = nc.declare_dram_parameter("beta1", [EMB], f32, isOutput=False)
    g2_d = nc.declare_dram_parameter("g2", [EMB], f32, isOutput=False)
    beta2_d = nc.declare_dram_parameter("beta2", [EMB], f32, isOutput=False)
    onescol_d = nc.declare_dram_parameter("onescol", [128, 1], f32, isOutput=False)
    onesrow_d = nc.declare_dram_parameter("onesrow", [1, 128], f32, isOutput=False)
    ident_d = nc.declare_dram_parameter("ident", [128, 128], f32, isOutput=False)
    bvf_d = nc.declare_dram_parameter("bvf", [128, DM], f32, isOutput=False)
    cwbc_d = nc.declare_dram_parameter("cwbc", [128, K], f32, isOutput=False)
    b2f_d = nc.declare_dram_parameter("b2f", [128, EMB], f32, isOutput=False)
    g2f_d = nc.declare_dram_parameter("g2f", [128, EMB], f32, isOutput=False)
    beta2f_d = nc.declare_dram_parameter("beta2f", [128, EMB], f32, isOutput=False)
    out_d = nc.declare_dram_parameter("out", [T, EMB], f32, isOutput=True)

    h1t_d = nc.dram_tensor("h1t", [EMB, T], f32)

    with tile.TileContext(nc) as tc:
        constp = tc.alloc_tile_pool(name="constp", bufs=1)

        ones_col = constp.tile([128, 1], f32r)
        nc.sync.dma_start(ones_col[:], onescol_d[:].bitcast(f32r))
        ones_row = constp.tile([1, 128], f32r)
        nc.sync.dma_start(ones_row[:], onesrow_d[:].bitcast(f32r))
        ident = constp.tile([128, 128], f32r)
        nc.sync.dma_start(ident[:], ident_d[:].bitcast(f32r))
        epsP = constp.tile([128, 1], f32)
        nc.gpsimd.memset(epsP[:], EPS)

        bqP = constp.tile([128, 8], f32)
        nc.sync.dma_start(bqP[:], bq_d.rearrange("(t p) -> p t", p=128))
        bkP = constp.tile([128, 8], f32)
        nc.sync.dma_start(bkP[:], bk_d.rearrange("(t p) -> p t", p=128))
        boP = constp.tile([128, 8], f32)
        nc.sync.dma_start(boP[:], bo_d.rearrange("(t p) -> p t", p=128))
        b1P = constp.tile([128, 40], f32)
        nc.sync.dma_start(b1P[:], b1_d.rearrange("(t p) -> p t", p=128))
        g1P = constp.tile([128, 10], f32)
        nc.sync.dma_start(g1P[:], g1_d.rearrange("(t p) -> p t", p=128))
        beta1P = constp.tile([128, 10], f32)
        nc.sync.dma_start(beta1P[:], beta1_d.rearrange("(t p) -> p t", p=128))
        maskP = constp.tile([128, 8], f32)
        nc.sync.dma_start(maskP[:], maskf_d.rearrange("(t p) -> p t", p=128))
        seqP = constp.tile([128, 8], f32)
        nc.sync.dma_start(seqP[:], seqf_d.rearrange("(t p) -> p t", p=128))
        seq_row = constp.tile([1, T], f32)
        nc.sync.dma_start(seq_row[:], seqf_d.rearrange("(a t) -> a t", a=1))
        # pre-broadcast constants shipped from host
        bvF = constp.tile([128, DM], f32)
        nc.sync.dma_start(bvF[:], bvf_d[:])
        convw_bc = constp.tile([128, K], f32)
        nc.sync.dma_start(convw_bc[:], cwbc_d[:])

        # ---------------- persistent activations ----------------
        xtp = tc.alloc_tile_pool(name="xtp", bufs=1)
        xt = xtp.tile([128, NKE, T], f32r)
        for k in range(NKE):
            nc.sync.dma_start(xt[:, k, :],
                              xt_d[k * 128:(k + 1) * 128, :].bitcast(f32r))

        attp = tc.alloc_tile_pool(name="attp", bufs=1)
        attT = attp.tile([128, NKD, T], f32r)

        qkvp = tc.alloc_tile_pool(name="qkvp", bufs=1)
        vaug = qkvp.tile([128, NT, H * HV], f32r)
        qt = qkvp.tile([128, NKD, T], f32r)
        kt = qkvp.tile([128, NKD, T], f32r)

        # ---------------- V projection (row-major, masked, augmented) ----
        with (
            tc.tile_pool(name="wvp", bufs=3) as wvp,
            tc.tile_pool(name="vps", bufs=8, space="PSUM") as vps,
            tc.tile_pool(name="vtmp", bufs=3) as vtmp,
        ):
            for n in range(2):
                pss = [vps.tile([128, 512], f32, name=f"vps{i}", tag="vps") for i in range(NT)]
                for k in range(NKD):
                    wvt = wvp.tile([128, 512], f32r)
                    nc.sync.dma_start(
                        wvt[:],
                        wv_d[k * 128:(k + 1) * 128, n * 512:(n + 1) * 512].bitcast(f32r),
                    )
                    for i in range(NT):
                        _mm(nc, pss[i][:], xt[:, 2 + k, i * 128:(i + 1) * 128],
                            wvt[:], k == 0, k == NKD - 1)
                for i in range(NT):
                    tmp = vtmp.tile([128, 512], f32)
                    nc.vector.tensor_add(tmp[:], pss[i][:], bvF[:, n * 512:(n + 1) * 512])
                    dest = vaug[:, i, :].rearrange("p (h c) -> p h c", c=HV)
                    dest = dest[:, n * 8:(n + 1) * 8, 0:DH]
                    nc.vector.tensor_scalar_mul(dest, tmp[:], maskP[:, i:i + 1])
            # mask columns (col 64 of each head slot)
            for i in range(NT):
                mcols = vaug[:, i, :].rearrange("p (h c) -> p h c", c=HV)[:, :, DH:HV]
                mcols = mcols.rearrange("p h c -> p (h c)")
                nc.vector.tensor_copy(mcols, maskP[:, i:i + 1].to_broadcast([128, H]))

        # ---------------- Q/K projections (feature-major) ----------------
        if phase >= 2:
         with (
             tc.tile_pool(name="wqp", bufs=3) as wqp,
             tc.tile_pool(name="qps", bufs=4, space="PSUM") as qps,
         ):
             for wdram, dst, biasP in ((wqr_d, qt, bqP), (wkr_d, kt, bkP)):
                 for m in range(8):
                     wt = wqp.tile([128, 8, 128], f32r, tag="wt")
                     nc.sync.dma_start(wt[:], wdram[m].rearrange("k p q -> p k q").bitcast(f32r))
                     ps = qps.tile([128, 2, 512], f32)
                     for n in range(2):
                         for k in range(NKD):
                             _mm(nc, ps[:, n, :], wt[:, k, :],
                                 xt[:, 2 + k, n * 512:(n + 1) * 512], k == 0, k == NKD - 1)
                     nc.scalar.activation(
                         dst[:, m, :], ps.rearrange("p a b -> p (a b)"), AF.Identity,
                         bias=biasP[:, m:m + 1],
                     )

        # ---------------- attention (two heads interleaved) ----------------
        if phase >= 3:
          with (
              tc.tile_pool(name="upool", bufs=3) as upool,
              tc.tile_pool(name="normp", bufs=2) as normp,
              tc.tile_pool(name="sps", bufs=2, space="PSUM") as spsp,
              tc.tile_pool(name="aps", bufs=1, space="PSUM") as apsp,
          ):
            NH = H if phase >= 4 else 2
            norm_tiles = {}

            def finalize(pair):
                # broadcast 1/norm along partitions via PE outer, scale in place
                for h in pair:
                    prow = (h % 2) * 64
                    ktile = h // 2
                    rps = spsp.tile([64, 2, 512], f32, name="rps", tag="sps")
                    for c in range(2):
                        _mm(nc, rps[:, c, :], ones_row[:, 0:64],
                            norm_tiles[h][:, c * 512:(c + 1) * 512], True, True)
                    dsth = attT[prow:prow + 64, ktile, :]
                    for c in range(2):
                        nc.vector.tensor_mul(
                            dsth[:, c * 512:(c + 1) * 512],
                            dsth[:, c * 512:(c + 1) * 512].bitcast(f32),
                            rps[:, c, :],
                        )

            prev_pair = None
            for hp in range(NH // 2):
                heads = (2 * hp, 2 * hp + 1)
                apss = {}
                for h in heads:
                    apss[h] = apsp.tile([HV, 2, 512], f32,
                                        name=f"aps{h % 2}", tag=f"aps{h % 2}")
                for jt in range(NT):
                    for h in heads:
                        prow = (h % 2) * 64
                        ktile = h // 2
                        sps = spsp.tile([128, 2, 512], f32, name="sps", tag="sps")
                        klhs = kt[prow:prow + 64, ktile, jt * 128:(jt + 1) * 128]
                        for c in range(2):
                            _mm(nc, sps[:, c, :], klhs,
                                qt[prow:prow + 64, ktile, c * 512:(c + 1) * 512],
                                True, True)
                        u = upool.tile([128, T], f32r, name="u", tag="u")
                        nc.scalar.activation(
                            u[:], sps.rearrange("p a b -> p (a b)"), AF.Exp,
                            scale=0.125,
                        )
                        vlhs = vaug[:, jt, h * HV:(h + 1) * HV]
                        for c in range(2):
                            _mm(nc, apss[h][:, c, :], vlhs,
                                u[:, c * 512:(c + 1) * 512], jt == 0, jt == NT - 1)
                for h in heads:
                    prow = (h % 2) * 64
                    ktile = h // 2
                    nt = normp.tile([1, T], f32r, name=f"nt{h % 2}", tag=f"nt{h % 2}")
                    norm_tiles[h] = nt
                    with nc.allow_low_precision(reason="fp32r softmax normalizer"):
                        nc.vector.reciprocal(
                            nt[:],
                            apss[h][DH:HV, :, :].rearrange("p a b -> p (a b)"),
                        )
                    # evict unnormalized attention; normalized in place in finalize()
                    nc.vector.tensor_copy(
                        attT[prow:prow + 64, ktile, :],
                        apss[h][0:DH, :, :].rearrange("p a b -> p (a b)"),
                    )
                if prev_pair is not None:
                    finalize(prev_pair)
                prev_pair = heads
            finalize(prev_pair)

        qkvp.release()

        # ---------------- h1pre = concat(conv, att@wo + bo) + x ----------
        if phase >= 5:
         h1p = tc.alloc_tile_pool(name="h1p", bufs=1)
         h1pre = h1p.tile([128, NKE, T], f32r)

         with tc.tile_pool(name="convp", bufs=2) as convp:
             for kb in range(2):
                 pad = convp.tile([128, T + 2], f32)
                 nc.gpsimd.memset(pad[:, 0:1], 0.0)
                 nc.gpsimd.memset(pad[:, T + 1:T + 2], 0.0)
                 nc.vector.tensor_copy(pad[:, 1:T + 1], xt[:, kb, :].bitcast(f32))
                 a1 = convp.tile([128, T], f32, tag="a1")
                 nc.vector.tensor_scalar_mul(a1[:], pad[:, 0:T], convw_bc[:, 0:1])
                 a2 = convp.tile([128, T], f32, tag="a2")
                 nc.vector.scalar_tensor_tensor(
                     a2[:], pad[:, 1:T + 1], convw_bc[:, 1:2], a1[:], OP.mult, OP.add
                 )
                 a3 = convp.tile([128, T], f32, tag="a3")
                 nc.vector.scalar_tensor_tensor(
                     a3[:], pad[:, 2:T + 2], convw_bc[:, 2:3], a2[:], OP.mult, OP.add
                 )
                 nc.vector.tensor_add(h1pre[:, kb, :], a3[:], xt[:, kb, :].bitcast(f32))

         with (
             tc.tile_pool(name="wop", bufs=3) as wop,
             tc.tile_pool(name="ops", bufs=4, space="PSUM") as opsp,
         ):
             for m in range(8):
                 wt = wop.tile([128, 8, 128], f32r, tag="wo")
                 nc.sync.dma_start(wt[:], wor_d[m].rearrange("k p q -> p k q").bitcast(f32r))
                 for n in range(2):
                     ps = opsp.tile([128, 512], f32)
                     # residual1: I.T @ x-tile seeds the accumulator
                     _mm(nc, ps[:], ident[:], xt[:, 2 + m, n * 512:(n + 1) * 512],
                         True, False)
                     for k in range(NKD):
                         _mm(nc, ps[:], wt[:, k, :],
                             attT[:, k, n * 512:(n + 1) * 512], False, k == NKD - 1)
                     nc.scalar.activation(
                         h1pre[:, 2 + m, n * 512:(n + 1) * 512], ps[:], AF.Identity,
                         bias=boP[:, m:m + 1],
                     )

         # ---------------- LayerNorm 1 (feature axis = partitions) -------
         with (
             tc.tile_pool(name="sqp", bufs=3) as sqp,
             tc.tile_pool(name="vecp", bufs=1) as vecp,
             tc.tile_pool(name="lnps", bufs=1, space="PSUM") as lnps,
             tc.tile_pool(name="lnops", bufs=2, space="PSUM") as lnops,
         ):
             musum = lnps.tile([1, 2, 512], f32, tag="musum")
             sqsum = lnps.tile([1, 2, 512], f32, tag="sqsum")
             for k in range(NKE):
                 sq = sqp.tile([128, T], f32r)
                 nc.vector.tensor_mul(sq[:], h1pre[:, k, :], h1pre[:, k, :])
                 for c in range(2):
                     _mm(nc, musum[:, c, :], ones_col[:],
                         h1pre[:, k, c * 512:(c + 1) * 512], k == 0, k == NKE - 1)
                     _mm(nc, sqsum[:, c, :], ones_col[:],
                         sq[:, c * 512:(c + 1) * 512], k == 0, k == NKE - 1)
             mu = vecp.tile([1, T], f32r)
             nc.vector.tensor_scalar_mul(
                 mu[:], musum.rearrange("p a b -> p (a b)"), 1.0 / EMB
             )
             ex2 = vecp.tile([1, T], f32)
             nc.vector.tensor_scalar_mul(
                 ex2[:], sqsum.rearrange("p a b -> p (a b)"), 1.0 / EMB
             )
             var = vecp.tile([1, T], f32)
             nc.vector.tensor_mul(var[:], mu.bitcast(f32)[:], mu.bitcast(f32)[:])
             nc.vector.tensor_sub(var[:], ex2[:], var[:])
             sd = vecp.tile([1, T], f32)
             nc.scalar.activation(sd[:], var[:], AF.Sqrt, bias=epsP[0:1, :])
             rs = vecp.tile([1, T], f32r)
             with nc.allow_low_precision(reason="fp32r LN1 inv-std"):
                 nc.vector.reciprocal(rs[:], sd[:])
             nc.vector.tensor_mul(rs[:], rs[:], seq_row.bitcast(f32r)[:])  # fold seq_mask
             muF = vecp.tile([128, T], f32, tag="muF")
             rsF = vecp.tile([128, T], f32, tag="rsF")
             for c in range(2):
                 pmu = lnops.tile([128, 512], f32)
                 _mm(nc, pmu[:], ones_row[:], mu[:, c * 512:(c + 1) * 512], True, True)
                 nc.scalar.activation(muF[:, c * 512:(c + 1) * 512], pmu[:], AF.Copy)
                 prs = lnops.tile([128, 512], f32)
                 _mm(nc, prs[:], ones_row[:], rs[:, c * 512:(c + 1) * 512], True, True)
                 nc.scalar.activation(rsF[:, c * 512:(c + 1) * 512], prs[:], AF.Copy)
             for k in range(NKE):
                 t1 = sqp.tile([128, T], f32, tag="t1")
                 nc.vector.tensor_sub(t1[:], h1pre[:, k, :].bitcast(f32), muF[:])
                 t2 = sqp.tile([128, T], f32, tag="t2")
                 nc.vector.tensor_mul(t2[:], t1[:], rsF[:])
                 t3 = sqp.tile([128, T], f32r, tag="t3")
                 nc.scalar.activation(
                     t3[:], t2[:], AF.Identity,
                     bias=beta1P[:, k:k + 1], scale=g1P[:, k:k + 1],
                 )
                 nc.sync.dma_start(h1t_d[k * 128:(k + 1) * 128, :].bitcast(f32r), t3[:])

         h1p.release()
        attp.release()
        xtp.release()

        # ---------------- FFN + LayerNorm 2, in two T-halves -------------
        if phase < 6:
            with tc.tile_pool(name="dummy", bufs=1) as dum:
                z = dum.tile([128, EMB], f32)
                nc.gpsimd.memset(z[:], 0.0)
                for t in range(NT):
                    nc.sync.dma_start(out_d[t * 128:(t + 1) * 128, :], z[:])
            constp.release()
            return nc
        ffnc = tc.alloc_tile_pool(name="ffnc", bufs=1)
        b2F = ffnc.tile([128, EMB], f32)
        nc.sync.dma_start(b2F[:], b2f_d[:])
        g2F = ffnc.tile([128, EMB], f32)
        nc.sync.dma_start(g2F[:], g2f_d[:])
        beta2F = ffnc.tile([128, EMB], f32)
        nc.sync.dma_start(beta2F[:], beta2f_d[:])

        ffp = tc.alloc_tile_pool(name="ffp", bufs=1)
        h1full = ffp.tile([128, NKE, T], f32r)
        for k in range(NKE):
            nc.sync.dma_start(h1full[:, k, :],
                              h1t_d[k * 128:(k + 1) * 128, :].bitcast(f32r))
        out2acc = ffp.tile([128, NT, EMB], f32)
        NSL = ((0, 512), (512, 512), (1024, 256))
        with (
            tc.tile_pool(name="w1p", bufs=3) as w1p,
            tc.tile_pool(name="w2p", bufs=6) as w2p,
            tc.tile_pool(name="blkp", bufs=2) as blkp,
            tc.tile_pool(name="ps1", bufs=2, space="PSUM") as ps1,
            tc.tile_pool(name="ps2", bufs=3, space="PSUM") as ps2,
            tc.tile_pool(name="ln2p", bufs=1) as ln2p,
        ):
            for blk in range(10):
                ffb = blkp.tile([128, 4, T], f32r, tag="ffb")
                for mi in range(4):
                    m = blk * 4 + mi
                    w1t = w1p.tile([128, 10, 128], f32r, tag="w1t")
                    nc.sync.dma_start(w1t[:], w1r_d[m].rearrange("k p q -> p k q").bitcast(f32r))
                    ps = ps1.tile([128, 2, 512], f32)
                    for k in range(NKE):
                        for c in range(2):
                            _mm(nc, ps[:, c, :], w1t[:, k, :],
                                h1full[:, k, c * 512:(c + 1) * 512],
                                k == 0, k == NKE - 1)
                    nc.scalar.activation(
                        ffb[:, mi, :], ps.rearrange("p a b -> p (a b)"),
                        AF.Relu, bias=b1P[:, m:m + 1],
                    )
                w2ts = []
                for ki in range(4):
                    k = blk * 4 + ki
                    w2t = w2p.tile([128, EMB], f32r, name=f"w2t{ki}", tag="w2t")
                    nc.sync.dma_start(
                        w2t[:], w2_d[k * 128:(k + 1) * 128, :].bitcast(f32r)
                    )
                    w2ts.append(w2t)
                for t in range(NT):
                    for n, (nbase, nsz) in enumerate(NSL):
                        pso = ps2.tile([128, 512], f32, name="pso", tag="pso")
                        for ki in range(4):
                            _mm(nc, pso[:, 0:nsz],
                                ffb[:, ki, t * 128:(t + 1) * 128],
                                w2ts[ki][:, nbase:nbase + nsz],
                                ki == 0, ki == 3)
                            if blk == 0 and ki == 0:
                                # residual2 via identity moving operand
                                for kb in range(nbase // 128, (nbase + nsz) // 128):
                                    _mm(nc, pso[:, kb * 128 - nbase:kb * 128 - nbase + 128],
                                        h1full[:, kb, t * 128:(t + 1) * 128], ident[:],
                                        False, False)
                        dst = out2acc[:, t, nbase:nbase + nsz]
                        if blk == 0:
                            nc.vector.tensor_add(dst, pso[:, 0:nsz],
                                                 b2F[:, nbase:nbase + nsz])
                        else:
                            nc.vector.tensor_add(dst, dst, pso[:, 0:nsz])
            # LayerNorm 2 (row-major per time tile) + store
            for gt in range(NT):
                o = out2acc[:, gt, :]
                rsum = ln2p.tile([128, 1], f32, tag="rsum")
                nc.vector.reduce_sum(rsum[:], o, axis=mybir.AxisListType.X)
                muv = ln2p.tile([128, 1], f32, tag="muv")
                nc.vector.tensor_scalar_mul(muv[:], rsum[:], 1.0 / EMB)
                cen = ln2p.tile([128, EMB], f32, tag="cen")
                nc.vector.tensor_scalar_sub(cen[:], o, muv[:])
                sqv = ln2p.tile([128, EMB], f32, tag="sqv")
                nc.vector.tensor_mul(sqv[:], cen[:], cen[:])
                vv = ln2p.tile([128, 1], f32, tag="vv")
                nc.vector.reduce_sum(vv[:], sqv[:], axis=mybir.AxisListType.X)
                nc.vector.tensor_scalar_mul(vv[:], vv[:], 1.0 / EMB)
                sdv = ln2p.tile([128, 1], f32, tag="sdv")
                nc.scalar.activation(sdv[:], vv[:], AF.Sqrt, bias=epsP[:])
                rv = ln2p.tile([128, 1], f32, tag="rv")
                nc.vector.reciprocal(rv[:], sdv[:])
                nc.vector.tensor_mul(rv[:], rv[:], seqP[:, gt:gt + 1])
                t5 = ln2p.tile([128, EMB], f32, tag="t5")
                nc.vector.scalar_tensor_tensor(
                    t5[:], cen[:], rv[:], g2F[:], OP.mult, OP.mult
                )
                t6 = ln2p.tile([128, EMB], f32, tag="sqv2")
                nc.vector.tensor_add(t6[:], t5[:], beta2F[:])
                nc.sync.dma_start(out_d[gt * 128:(gt + 1) * 128, :], t6[:])
        ffp.release()
        ffnc.release()
        constp.release()

    return nc


def _split_matmul_waits(bj: bytes) -> bytes:
    """Walrus codegen allows only one sync-wait on Matmult/DMACopy
    instructions; hoist extra waits onto a preceding EventSemaphore."""
    d = json.loads(bj)
    n = 0
    for f in d["functions"]:
        for blk in f["blocks"]:
            out = []
            for inst in blk["instructions"]:
                si = inst.get("sync_info")
                if (si and si.get("on_wait") and len(si["on_wait"]) >= 2
                        and inst.get("opcode") != "EventSemaphore"):
                    waits = si["on_wait"]
                    for w in waits[:-1]:
                        out.append({
                            "debug": inst.get("debug"),
                            "engine": inst["engine"],
                            "ins": [],
                            "outs": [],
                            "name": f"waitfix_{n}",
                            "opcode": "EventSemaphore",
                            "sync_info": {"on_update": [], "on_wait": [w]},
                        })
                        n += 1
                    si["on_wait"] = waits[-1:]
                out.append(inst)
            blk["instructions"] = out
    return json.dumps(d).encode()


_NC_CACHE = None


def _get_nc():
    global _NC_CACHE
    if _NC_CACHE is None:
        nc = build_nc()
        orig = nc.to_json_bytes
        nc.to_json_bytes = lambda: _split_matmul_waits(orig())
        _NC_CACHE = nc
    return _NC_CACHE


def _prep_core_inputs(x_b, mask_b, seq_b, conv_w, wq, bq, wk, bk, wv, bv, wo, bo,
                      w1, b1, w2, b2, g1, beta1, g2, beta2):
    f = np.float32
    return {
        "xt": np.ascontiguousarray(x_b.T, dtype=f),
        "maskf": np.ascontiguousarray((mask_b == 0).astype(f)),
        "seqf": np.ascontiguousarray(seq_b.astype(f)),
        "convw": np.ascontiguousarray(conv_w.reshape(K).astype(f)),
        "wqr": np.ascontiguousarray(wq.reshape(8, 128, 8, 128).transpose(2, 0, 1, 3)),
        "wkr": np.ascontiguousarray(wk.reshape(8, 128, 8, 128).transpose(2, 0, 1, 3)),
        "wv": np.ascontiguousarray(wv.astype(f)),
        "wor": np.ascontiguousarray(wo.reshape(8, 128, 8, 128).transpose(2, 0, 1, 3)),
        "w1r": np.ascontiguousarray(w1.reshape(10, 128, 40, 128).transpose(2, 0, 1, 3)),
        "w2": np.ascontiguousarray(w2.astype(f)),
        "onescol": np.ones((128, 1), f),
        "onesrow": np.ones((1, 128), f),
        "ident": np.eye(128, dtype=f),
        "bvf": np.ascontiguousarray(np.tile(bv.astype(f)[None, :], (128, 1))),
        "cwbc": np.ascontiguousarray(np.tile(conv_w.reshape(K).astype(f)[None, :], (128, 1))),
        "b2f": np.ascontiguousarray(np.tile(b2.astype(f)[None, :], (128, 1))),
        "g2f": np.ascontiguousarray(np.tile(g2.astype(f)[None, :], (128, 1))),
        "beta2f": np.ascontiguousarray(np.tile(beta2.astype(f)[None, :], (128, 1))),
        "bq": np.ascontiguousarray(bq.astype(f)),
        "bk": np.ascontiguousarray(bk.astype(f)),
        "bv": np.ascontiguousarray(bv.astype(f)),
        "bo": np.ascontiguousarray(bo.astype(f)),
        "b1": np.ascontiguousarray(b1.astype(f)),
        "b2": np.ascontiguousarray(b2.astype(f)),
        "g1": np.ascontiguousarray(g1.astype(f)),
        "beta1": np.ascontiguousarray(beta1.astype(f)),
        "g2": np.ascontiguousarray(g2.astype(f)),
        "beta2": np.ascontiguousarray(beta2.astype(f)),
    }


def kernel(x, att_mask, seq_mask, conv_w, wq, bq, wk, bk, wv, bv, wo, bo,
           w1, b1, w2, b2, g1, beta1, g2, beta2, _trace=False):
    from concourse.bass_utils import run_bass_kernel_spmd

    nc = _get_nc()
    x = np.asarray(x, dtype=np.float32)
    in_maps = []
    for b in range(B):
        in_maps.append(_prep_core_inputs(
            x[b], np.asarray(att_mask)[b], np.asarray(seq_mask)[b, :, 0],
            np.asarray(conv_w), np.asarray(wq), np.asarray(bq), np.asarray(wk),
            np.asarray(bk), np.asarray(wv), np.asarray(bv), np.asarray(wo),
            np.asarray(bo), np.asarray(w1), np.asarray(b1), np.asarray(w2),
            np.asarray(b2), np.asarray(g1), np.asarray(beta1), np.asarray(g2),
            np.asarray(beta2)))
    res = run_bass_kernel_spmd(nc, in_maps, list(range(B)), trace=_trace)
    out = np.stack([res.results[i]["out"] for i in range(B)], axis=0)
    if _trace:
        return out, res
    return out



# revision 10
# speedup vs baseline: 1.1969x; 1.1969x over previous
"""Trainium2 Bass/Tile kernel for nn_EncoderLayer (dense transformer block).

Strategy: pure data-parallel over batch (B=8 -> 1 batch element per core, no
collectives). Per core, activations are kept feature-major ([D, T]) in bf16
(same PE matmul rate as fp32r, half the DMA/SBUF, 2x DVE). Attention folds
the key mask into the exp bias (per-partition = per-key) and appends a ones
column to V so P@V yields the softmax normalizer for free; the attention
inner loop is software-pipelined (scores lag PV by one time-tile) so the
Activation engine's exp stream never starves. h1 stays resident in SBUF
(no DRAM round trip). FFN1 output (all 40 row-tiles) stays resident in bf16,
so FFN2 accumulates entirely in PSUM feature-major (no SBUF accumulation
adds) and LayerNorm2 statistics stream on the PE during the k-sweep; the
final transpose to row-major is done with PE transpose matmuls. Partition
broadcasts for LN run on the otherwise-idle GpSimd/Pool engine.
"""

import json
import sys

if "/opt/trn_rl_repo" not in sys.path:
    sys.path.insert(0, "/opt/trn_rl_repo")

import numpy as np
import ml_dtypes

import concourse.bass as bass
import concourse.mybir as mybir
import concourse.tile as tile
from concourse import library_config

B, T, CC, DM, H, DH, DFF, K = 8, 1024, 256, 1024, 16, 64, 5120, 3
EMB = CC + DM  # 1280
EPS = 1e-6
f32 = mybir.dt.float32
bf16 = mybir.dt.bfloat16
AF = mybir.ActivationFunctionType
OP = mybir.AluOpType

NT = T // 128          # 8 time tiles
NKE = EMB // 128       # 10 embed k-tiles
NKD = DM // 128        # 8 d_model k-tiles
NMF = DFF // 128       # 40 d_ff tiles
HV = DH + 1            # 65: per-head V columns + normalizer ones column
MASK_NEG = -60000.0    # exp(-60000 + s/8) == 0.0 in f32


def _mm(nc, out, lhsT, rhs, start, stop):
    nc.tensor.matmul(out, lhsT, rhs, start=start, stop=stop)


def build_nc():
    nc = bass.Bass()

    xt_d = nc.declare_dram_parameter("xt", [EMB, T], bf16, isOutput=False)
    wv_d = nc.declare_dram_parameter("wv", [DM, DM], bf16, isOutput=False)
    wqr_d = nc.declare_dram_parameter("wqr", [8, 128, 8, 128], bf16, isOutput=False)
    wkr_d = nc.declare_dram_parameter("wkr", [8, 128, 8, 128], bf16, isOutput=False)
    wor_d = nc.declare_dram_parameter("wor", [8, 128, 8, 128], bf16, isOutput=False)
    w1r_d = nc.declare_dram_parameter("w1r", [40, 128, 10, 128], bf16, isOutput=False)
    w2r_d = nc.declare_dram_parameter("w2r", [10, 128, 40, 128], bf16, isOutput=False)
    bvf_d = nc.declare_dram_parameter("bvf", [128, DM], bf16, isOutput=False)
    mbias_d = nc.declare_dram_parameter("mbias", [128, 8], f32, isOutput=False)
    bqp_d = nc.declare_dram_parameter("bqp", [128, 8], f32, isOutput=False)
    bkp_d = nc.declare_dram_parameter("bkp", [128, 8], f32, isOutput=False)
    bop_d = nc.declare_dram_parameter("bop", [128, 8], f32, isOutput=False)
    b1p_d = nc.declare_dram_parameter("b1p", [128, 40], f32, isOutput=False)
    b2p_d = nc.declare_dram_parameter("b2p", [128, 10], f32, isOutput=False)
    g1p_d = nc.declare_dram_parameter("g1p", [128, 10], f32, isOutput=False)
    beta1p_d = nc.declare_dram_parameter("beta1p", [128, 10], f32, isOutput=False)
    g2p_d = nc.declare_dram_parameter("g2p", [128, 10], f32, isOutput=False)
    beta2p_d = nc.declare_dram_parameter("beta2p", [128, 10], f32, isOutput=False)
    cwbc_d = nc.declare_dram_parameter("cwbc", [128, K], f32, isOutput=False)
    seqrow_d = nc.declare_dram_parameter("seqrow", [1, T], f32, isOutput=False)
    onescol_d = nc.declare_dram_parameter("onescol", [128, 1], bf16, isOutput=False)
    ident_d = nc.declare_dram_parameter("ident", [128, 128], bf16, isOutput=False)
    out_d = nc.declare_dram_parameter("out", [T, EMB], f32, isOutput=True)

    with tile.TileContext(nc) as tc:
        nc.gpsimd.load_library(library_config.attnmlp)

        # ---------------- persistent pools (alloc in reverse-death order) ---
        constp = tc.alloc_tile_pool(name="constp", bufs=1)
        h1p = tc.alloc_tile_pool(name="h1p", bufs=1)
        h1 = h1p.tile([128, NKE, T], bf16)
        h1prep = tc.alloc_tile_pool(name="h1prep", bufs=1)
        h1pre = h1prep.tile([128, NKE, T], bf16)
        attp = tc.alloc_tile_pool(name="attp", bufs=1)
        attT = attp.tile([128, NKD, T], bf16)
        xtp = tc.alloc_tile_pool(name="xtp", bufs=1)
        xt = xtp.tile([128, NKE, T], bf16)
        for k in range(2, NKE):  # V/QK feature tiles first
            nc.sync.dma_start(xt[:, k, :], xt_d[k * 128:(k + 1) * 128, :])
        bvF = constp.tile([128, DM], bf16)
        nc.sync.dma_start(bvF[:], bvf_d[:])
        mbias = constp.tile([128, 8], f32)
        nc.sync.dma_start(mbias[:], mbias_d[:])
        bqP = constp.tile([128, 8], f32)
        nc.sync.dma_start(bqP[:], bqp_d[:])
        bkP = constp.tile([128, 8], f32)
        nc.sync.dma_start(bkP[:], bkp_d[:])
        boP = constp.tile([128, 8], f32)
        nc.sync.dma_start(boP[:], bop_d[:])
        b1P = constp.tile([128, 40], f32)
        nc.sync.dma_start(b1P[:], b1p_d[:])
        b2P = constp.tile([128, 10], f32)
        nc.sync.dma_start(b2P[:], b2p_d[:])
        g1P = constp.tile([128, 10], f32)
        nc.sync.dma_start(g1P[:], g1p_d[:])
        beta1P = constp.tile([128, 10], f32)
        nc.sync.dma_start(beta1P[:], beta1p_d[:])
        g2P = constp.tile([128, 10], f32)
        nc.sync.dma_start(g2P[:], g2p_d[:])
        beta2P = constp.tile([128, 10], f32)
        nc.sync.dma_start(beta2P[:], beta2p_d[:])
        cwbc = constp.tile([128, K], f32)
        nc.sync.dma_start(cwbc[:], cwbc_d[:])
        seq_row = constp.tile([1, T], f32)
        nc.sync.dma_start(seq_row[:], seqrow_d[:])
        ones_col = constp.tile([128, 1], bf16)
        nc.sync.dma_start(ones_col[:], onescol_d[:])
        ident = constp.tile([128, 128], bf16)
        nc.sync.dma_start(ident[:], ident_d[:])
        epsP = constp.tile([128, 1], f32)
        nc.gpsimd.memset(epsP[:], EPS)

        for k in range(2):  # conv feature tiles
            nc.sync.dma_start(xt[:, k, :], xt_d[k * 128:(k + 1) * 128, :])

        vp = tc.alloc_tile_pool(name="vp", bufs=1)
        vaug = vp.tile([128, NT, H * HV], bf16)
        # normalizer ones column (col DH of each head slot)
        ocols = vaug.rearrange("p j (h c) -> p (j h) c", c=HV)[:, :, DH:HV]
        nc.gpsimd.memset(ocols, 1.0)

        qkp = tc.alloc_tile_pool(name="qkp", bufs=1)
        qt = qkp.tile([128, NKD, T], bf16)
        kt = qkp.tile([128, NKD, T], bf16)

        # ---------------- V projection (row-major, bias, augmented) --------
        with (
            tc.tile_pool(name="wvp", bufs=3) as wvp,
            tc.tile_pool(name="vps", bufs=8, space="PSUM") as vps,
        ):
            for n in range(2):
                pss = [vps.tile([128, 512], f32, name=f"vps{i}", tag="vps")
                       for i in range(NT)]
                for k in range(NKD):
                    wvt = wvp.tile([128, 512], bf16)
                    nc.sync.dma_start(
                        wvt[:], wv_d[k * 128:(k + 1) * 128, n * 512:(n + 1) * 512])
                    for i in range(NT):
                        _mm(nc, pss[i][:], xt[:, 2 + k, i * 128:(i + 1) * 128],
                            wvt[:], k == 0, k == NKD - 1)
                for i in range(NT):
                    dest = vaug[:, i, :].rearrange("p (h c) -> p h c", c=HV)
                    dest = dest[:, n * 8:(n + 1) * 8, 0:DH]
                    nc.vector.tensor_add(dest, pss[i][:],
                                         bvF[:, n * 512:(n + 1) * 512])

        # ---------------- Q/K projections (feature-major) ------------------
        with (
            tc.tile_pool(name="wqp", bufs=3) as wqp,
            tc.tile_pool(name="qps", bufs=2, space="PSUM") as qps,
        ):
            for wdram, dst, biasP in ((wqr_d, qt, bqP), (wkr_d, kt, bkP)):
                for m in range(8):
                    wt = wqp.tile([128, 8, 128], bf16, tag="wt")
                    nc.sync.dma_start(wt[:], wdram[m])
                    ps = qps.tile([128, 2, 512], f32)
                    for n in range(2):
                        for k in range(NKD):
                            _mm(nc, ps[:, n, :], wt[:, k, :],
                                xt[:, 2 + k, n * 512:(n + 1) * 512],
                                k == 0, k == NKD - 1)
                    nc.vector.tensor_scalar_add(
                        dst[:, m, :], ps.rearrange("p a b -> p (a b)"),
                        biasP[:, m:m + 1])

        # ---------------- attention (single head, lag-1 pipelined) ---------
        with (
            tc.tile_pool(name="upool", bufs=3) as upool,
            tc.tile_pool(name="normp", bufs=2) as normp,
            tc.tile_pool(name="bcp", bufs=2) as bcp,
            tc.tile_pool(name="sps", bufs=2, space="PSUM") as spsp,
            tc.tile_pool(name="aps", bufs=2, space="PSUM") as apsp,
        ):
            def scores_exp(h, jt):
                prow = (h % 2) * 64
                ktile = h // 2
                sps = spsp.tile([128, 2, 512], f32, name="sps", tag="sps")
                klhs = kt[prow:prow + 64, ktile, jt * 128:(jt + 1) * 128]
                for c in range(2):
                    _mm(nc, sps[:, c, :], klhs,
                        qt[prow:prow + 64, ktile, c * 512:(c + 1) * 512],
                        True, True)
                u = upool.tile([128, T], bf16, name="u", tag="u")
                nc.scalar.activation(
                    u[:], sps.rearrange("p a b -> p (a b)"), AF.Exp,
                    scale=0.125, bias=mbias[:, jt:jt + 1])
                return u

            def pv(h, jt, u, aps):
                vlhs = vaug[:, jt, h * HV:(h + 1) * HV]
                for c in range(2):
                    _mm(nc, aps[:, c, :], vlhs,
                        u[:, c * 512:(c + 1) * 512], jt == 0, jt == NT - 1)

            def evacuate(h, aps):
                # 1/normalizer; unnormalized attention rows -> attT
                prow = (h % 2) * 64
                ktile = h // 2
                nt_ = normp.tile([1, T], bf16, name="nt", tag="nt")
                with nc.allow_low_precision(reason="bf16 softmax normalizer"):
                    nc.vector.reciprocal(
                        nt_[:], aps[DH:HV, :, :].rearrange("p a b -> p (a b)"))
                nc.vector.tensor_copy(
                    attT[prow:prow + 64, ktile, :],
                    aps[0:DH, :, :].rearrange("p a b -> p (a b)"))
                return nt_

            def finalize(h, nt_):
                prow = (h % 2) * 64
                ktile = h // 2
                bc = bcp.tile([64, T], bf16, name="bc", tag="bc")
                nc.gpsimd.partition_broadcast(bc[:], nt_[0:1, :])
                nc.vector.tensor_mul(
                    attT[prow:prow + 64, ktile, :],
                    attT[prow:prow + 64, ktile, :], bc[:])

            pending = None  # (h, norm_tile) awaiting broadcast+scale
            for h in range(H):
                aps = apsp.tile([HV, 2, 512], f32, name="aps", tag="aps")
                us = [scores_exp(h, 0), scores_exp(h, 1)]
                for jt in range(NT):
                    if jt + 2 < NT:
                        us.append(scores_exp(h, jt + 2))
                    pv(h, jt, us[jt], aps)
                nt_ = evacuate(h, aps)
                if pending is not None:
                    finalize(*pending)
                pending = (h, nt_)
            finalize(*pending)

        qkp.release()
        vp.release()

        # ---------------- h1pre = concat(conv, att@wo + bo) + x ------------
        with (
            tc.tile_pool(name="convp", bufs=2) as convp,
            tc.tile_pool(name="wop", bufs=3) as wop,
            tc.tile_pool(name="ops", bufs=4, space="PSUM") as opsp,
            tc.tile_pool(name="lnps", bufs=1, space="PSUM") as lnps,
            tc.tile_pool(name="sqp", bufs=3) as sqp,
            tc.tile_pool(name="vecp", bufs=1) as vecp,
        ):
            musum = lnps.tile([1, 2, 512], f32, tag="musum")
            sqsum = lnps.tile([1, 2, 512], f32, tag="sqsum")

            def ln1_k(kb):
                sq = sqp.tile([128, T], bf16, tag="sq")
                nc.vector.tensor_mul(sq[:], h1pre[:, kb, :], h1pre[:, kb, :])
                for c in range(2):
                    _mm(nc, musum[:, c, :], ones_col[:],
                        h1pre[:, kb, c * 512:(c + 1) * 512], kb == 0, kb == NKE - 1)
                    _mm(nc, sqsum[:, c, :], ones_col[:],
                        sq[:, c * 512:(c + 1) * 512], kb == 0, kb == NKE - 1)

            # depthwise conv (DVE) on the first two feature tiles
            for kb in range(2):
                pad = convp.tile([128, T + 2], bf16, tag="pad")
                nc.gpsimd.memset(pad[:, 0:1], 0.0)
                nc.gpsimd.memset(pad[:, T + 1:T + 2], 0.0)
                nc.vector.tensor_copy(pad[:, 1:T + 1], xt[:, kb, :])
                a1 = convp.tile([128, T], bf16, tag="a1")
                nc.vector.tensor_scalar_mul(a1[:], pad[:, 0:T], cwbc[:, 0:1])
                a2 = convp.tile([128, T], bf16, tag="a2")
                nc.vector.scalar_tensor_tensor(
                    a2[:], pad[:, 1:T + 1], cwbc[:, 1:2], a1[:], OP.mult, OP.add)
                a3 = convp.tile([128, T], bf16, tag="a3")
                nc.vector.scalar_tensor_tensor(
                    a3[:], pad[:, 2:T + 2], cwbc[:, 2:3], a2[:], OP.mult, OP.add)
                nc.vector.tensor_add(h1pre[:, kb, :], a3[:], xt[:, kb, :])
                ln1_k(kb)

            # attention out-projection with residual seeded via identity
            for m in range(8):
                wt = wop.tile([128, 8, 128], bf16, tag="wo")
                nc.sync.dma_start(wt[:], wor_d[m])
                for n in range(2):
                    ps = opsp.tile([128, 512], f32)
                    _mm(nc, ps[:], ident[:], xt[:, 2 + m, n * 512:(n + 1) * 512],
                        True, False)
                    for k in range(NKD):
                        _mm(nc, ps[:], wt[:, k, :],
                            attT[:, k, n * 512:(n + 1) * 512], False, k == NKD - 1)
                    nc.scalar.activation(
                        h1pre[:, 2 + m, n * 512:(n + 1) * 512], ps[:], AF.Identity,
                        bias=boP[:, m:m + 1])
                ln1_k(2 + m)

            # LayerNorm 1 statistics + normalize (feature axis = partitions)
            mu = vecp.tile([1, T], f32, tag="mu")
            nc.vector.tensor_scalar_mul(
                mu[:], musum.rearrange("p a b -> p (a b)"), 1.0 / EMB)
            ex2 = vecp.tile([1, T], f32, tag="ex2")
            nc.vector.tensor_scalar_mul(
                ex2[:], sqsum.rearrange("p a b -> p (a b)"), 1.0 / EMB)
            var = vecp.tile([1, T], f32, tag="var")
            nc.vector.tensor_mul(var[:], mu[:], mu[:])
            nc.vector.tensor_sub(var[:], ex2[:], var[:])
            sd = vecp.tile([1, T], f32, tag="sd")
            nc.scalar.activation(sd[:], var[:], AF.Sqrt, bias=epsP[0:1, :])
            rs = vecp.tile([1, T], f32, tag="rs")
            nc.vector.reciprocal(rs[:], sd[:])
            nc.vector.tensor_mul(rs[:], rs[:], seq_row[:])  # fold seq_mask
            muB = vecp.tile([1, T], bf16, tag="muB")
            nc.vector.tensor_copy(muB[:], mu[:])
            rsB = vecp.tile([1, T], bf16, tag="rsB")
            with nc.allow_low_precision(reason="bf16 LN1 factors"):
                nc.vector.tensor_copy(rsB[:], rs[:])
            muF = vecp.tile([128, T], bf16, tag="muF")
            nc.gpsimd.partition_broadcast(muF[:], muB[0:1, :])
            rsF = vecp.tile([128, T], bf16, tag="rsF")
            nc.gpsimd.partition_broadcast(rsF[:], rsB[0:1, :])
            for kb in range(NKE):
                t1 = sqp.tile([128, T], bf16, tag="t1")
                nc.vector.tensor_sub(t1[:], h1pre[:, kb, :], muF[:])
                t2 = sqp.tile([128, T], bf16, tag="t2")
                nc.vector.tensor_mul(t2[:], t1[:], rsF[:])
                nc.scalar.activation(
                    h1[:, kb, :], t2[:], AF.Identity,
                    bias=beta1P[:, kb:kb + 1], scale=g1P[:, kb:kb + 1])

        xtp.release()
        attp.release()
        h1prep.release()

        # ---------------- FFN1: ffb[m] = relu(h1 @ w1 + b1), all resident --
        outp = tc.alloc_tile_pool(name="outp", bufs=1)
        oacc = outp.tile([128, NKE, T], bf16)
        ffbp = tc.alloc_tile_pool(name="ffbp", bufs=1)
        ffb = ffbp.tile([128, NMF, T], bf16)
        with (
            tc.tile_pool(name="w1p", bufs=3) as w1p,
            tc.tile_pool(name="ps1", bufs=3, space="PSUM") as ps1,
        ):
            for mf in range(NMF):
                w1t = w1p.tile([128, 10, 128], bf16, tag="w1t")
                nc.sync.dma_start(w1t[:], w1r_d[mf])
                ps = ps1.tile([128, 2, 512], f32)
                for k in range(NKE):
                    for c in range(2):
                        _mm(nc, ps[:, c, :], w1t[:, k, :],
                            h1[:, k, c * 512:(c + 1) * 512], k == 0, k == NKE - 1)
                nc.scalar.activation(
                    ffb[:, mf, :], ps.rearrange("p a b -> p (a b)"),
                    AF.Relu, bias=b1P[:, mf:mf + 1])

        # ---------------- FFN2 + LayerNorm 2 (feature-major) ---------------
        with (
            tc.tile_pool(name="w2p", bufs=2) as w2p,
            tc.tile_pool(name="ps2", bufs=2, space="PSUM") as ps2,
            tc.tile_pool(name="lnps2", bufs=1, space="PSUM") as lnps2,
            tc.tile_pool(name="sq2p", bufs=3) as sq2p,
            tc.tile_pool(name="vec2p", bufs=1) as vec2p,
        ):
            musum2 = lnps2.tile([1, 2, 512], f32, tag="musum2")
            sqsum2 = lnps2.tile([1, 2, 512], f32, tag="sqsum2")
            for e in range(NKE):
                w2t = w2p.tile([128, 40, 128], bf16, tag="w2t")
                nc.sync.dma_start(w2t[:], w2r_d[e])
                pso = ps2.tile([128, 2, 512], f32)
                for k in range(NMF):
                    for c in range(2):
                        _mm(nc, pso[:, c, :], w2t[:, k, :],
                            ffb[:, k, c * 512:(c + 1) * 512], k == 0, k == NMF - 1)
                # oacc[e] = (pso + b2) + h1[e]   (residual2)
                nc.vector.scalar_tensor_tensor(
                    oacc[:, e, :], pso.rearrange("p a b -> p (a b)"),
                    b2P[:, e:e + 1], h1[:, e, :], OP.add, OP.add)
                sq = sq2p.tile([128, T], bf16, tag="sq2")
                nc.vector.tensor_mul(sq[:], oacc[:, e, :], oacc[:, e, :])
                for c in range(2):
                    _mm(nc, musum2[:, c, :], ones_col[:],
                        oacc[:, e, c * 512:(c + 1) * 512], e == 0, e == NKE - 1)
                    _mm(nc, sqsum2[:, c, :], ones_col[:],
                        sq[:, c * 512:(c + 1) * 512], e == 0, e == NKE - 1)

            mu = vec2p.tile([1, T], f32, tag="mu2")
            nc.vector.tensor_scalar_mul(
                mu[:], musum2.rearrange("p a b -> p (a b)"), 1.0 / EMB)
            ex2 = vec2p.tile([1, T], f32, tag="ex22")
            nc.vector.tensor_scalar_mul(
                ex2[:], sqsum2.rearrange("p a b -> p (a b)"), 1.0 / EMB)
            var = vec2p.tile([1, T], f32, tag="var2")
            nc.vector.tensor_mul(var[:], mu[:], mu[:])
            nc.vector.tensor_sub(var[:], ex2[:], var[:])
            sd = vec2p.tile([1, T], f32, tag="sd2")
            nc.scalar.activation(sd[:], var[:], AF.Sqrt, bias=epsP[0:1, :])
            rs = vec2p.tile([1, T], f32, tag="rs2")
            nc.vector.reciprocal(rs[:], sd[:])
            nc.vector.tensor_mul(rs[:], rs[:], seq_row[:])
            muB = vec2p.tile([1, T], bf16, tag="muB2")
            nc.vector.tensor_copy(muB[:], mu[:])
            rsB = vec2p.tile([1, T], bf16, tag="rsB2")
            with nc.allow_low_precision(reason="bf16 LN2 factors"):
                nc.vector.tensor_copy(rsB[:], rs[:])
            muF = vec2p.tile([128, T], bf16, tag="muF2")
            nc.gpsimd.partition_broadcast(muF[:], muB[0:1, :])
            rsF = vec2p.tile([128, T], bf16, tag="rsF2")
            nc.gpsimd.partition_broadcast(rsF[:], rsB[0:1, :])
            for e in range(NKE):
                t1 = sq2p.tile([128, T], bf16, tag="t12")
                nc.vector.tensor_sub(t1[:], oacc[:, e, :], muF[:])
                t2 = sq2p.tile([128, T], bf16, tag="t22")
                nc.vector.tensor_mul(t2[:], t1[:], rsF[:])
                nc.scalar.activation(
                    oacc[:, e, :], t2[:], AF.Identity,
                    bias=beta2P[:, e:e + 1], scale=g2P[:, e:e + 1])

        # ---------------- transpose to row-major + store --------------------
        ffbp.release()
        with (
            tc.tile_pool(name="psT", bufs=2, space="PSUM") as psT,
            tc.tile_pool(name="obuf", bufs=2) as obuf,
        ):
            for tb in range(NT):
                pt = psT.tile([128, NKE, 128], bf16)
                for e in range(NKE):
                    nc.tensor.matmul(
                        pt[:, e, :], oacc[:, e, tb * 128:(tb + 1) * 128],
                        ident[:], start=True, stop=True, is_transpose=True)
                ob = obuf.tile([128, EMB], f32)
                nc.scalar.activation(
                    ob[:], pt.rearrange("p a b -> p (a b)"), AF.Identity)
                nc.sync.dma_start(out_d[tb * 128:(tb + 1) * 128, :], ob[:])

        outp.release()
        h1p.release()
        constp.release()

    return nc


def _split_matmul_waits(bj: bytes) -> bytes:
    """Walrus codegen allows only one sync-wait on Matmult/DMACopy
    instructions; hoist extra waits onto a preceding EventSemaphore."""
    d = json.loads(bj)
    n = 0
    for f in d["functions"]:
        for blk in f["blocks"]:
            out = []
            for inst in blk["instructions"]:
                si = inst.get("sync_info")
                if (si and si.get("on_wait") and len(si["on_wait"]) >= 2
                        and inst.get("opcode") != "EventSemaphore"):
                    waits = si["on_wait"]
                    for w in waits[:-1]:
                        out.append({
                            "debug": inst.get("debug"),
                            "engine": inst["engine"],
                            "ins": [],
                            "outs": [],
                            "name": f"waitfix_{n}",
                            "opcode": "EventSemaphore",
                            "sync_info": {"on_update": [], "on_wait": [w]},
                        })
                        n += 1
                    si["on_wait"] = waits[-1:]
                out.append(inst)
            blk["instructions"] = out
    return json.dumps(d).encode()


_NC_CACHE = None


def _get_nc():
    global _NC_CACHE
    if _NC_CACHE is None:
        nc = build_nc()
        orig = nc.to_json_bytes
        nc.to_json_bytes = lambda: _split_matmul_waits(orig())
        _NC_CACHE = nc
    return _NC_CACHE


def _prep_core_inputs(x_b, mask_b, seq_b, conv_w, wq, bq, wk, bk, wv, bv, wo, bo,
                      w1, b1, w2, b2, g1, beta1, g2, beta2):
    f = np.float32
    bf = ml_dtypes.bfloat16
    mask_b = np.asarray(mask_b)
    masked = (mask_b != 0).astype(f)  # reference: att_mask != 0 -> -1e9 score
    return {
        "xt": np.ascontiguousarray(x_b.T).astype(bf),
        "wv": np.ascontiguousarray(wv).astype(bf),
        "wqr": np.ascontiguousarray(
            wq.reshape(8, 128, 8, 128).transpose(2, 1, 0, 3)).astype(bf),
        "wkr": np.ascontiguousarray(
            wk.reshape(8, 128, 8, 128).transpose(2, 1, 0, 3)).astype(bf),
        "wor": np.ascontiguousarray(
            wo.reshape(8, 128, 8, 128).transpose(2, 1, 0, 3)).astype(bf),
        "w1r": np.ascontiguousarray(
            w1.reshape(10, 128, 40, 128).transpose(2, 1, 0, 3)).astype(bf),
        "w2r": np.ascontiguousarray(
            w2.reshape(40, 128, 10, 128).transpose(2, 1, 0, 3)).astype(bf),
        "bvf": np.tile(np.asarray(bv, f)[None, :], (128, 1)).astype(bf),
        "mbias": np.ascontiguousarray(
            (MASK_NEG * masked).reshape(8, 128).T.astype(f)),
        "bqp": np.ascontiguousarray(np.asarray(bq, f).reshape(8, 128).T),
        "bkp": np.ascontiguousarray(np.asarray(bk, f).reshape(8, 128).T),
        "bop": np.ascontiguousarray(np.asarray(bo, f).reshape(8, 128).T),
        "b1p": np.ascontiguousarray(np.asarray(b1, f).reshape(40, 128).T),
        "b2p": np.ascontiguousarray(np.asarray(b2, f).reshape(10, 128).T),
        "g1p": np.ascontiguousarray(np.asarray(g1, f).reshape(10, 128).T),
        "beta1p": np.ascontiguousarray(np.asarray(beta1, f).reshape(10, 128).T),
        "g2p": np.ascontiguousarray(np.asarray(g2, f).reshape(10, 128).T),
        "beta2p": np.ascontiguousarray(np.asarray(beta2, f).reshape(10, 128).T),
        "cwbc": np.tile(np.asarray(conv_w, f).reshape(K)[None, :], (128, 1)),
        "seqrow": np.ascontiguousarray(np.asarray(seq_b, f).reshape(1, T)),
        "onescol": np.ones((128, 1), bf),
        "ident": np.eye(128, dtype=f).astype(bf),
    }


def kernel(x, att_mask, seq_mask, conv_w, wq, bq, wk, bk, wv, bv, wo, bo,
           w1, b1, w2, b2, g1, beta1, g2, beta2, _trace=False):
    from concourse.bass_utils import run_bass_kernel_spmd

    nc = _get_nc()
    x = np.asarray(x, dtype=np.float32)
    in_maps = []
    for b in range(B):
        in_maps.append(_prep_core_inputs(
            x[b], np.asarray(att_mask)[b], np.asarray(seq_mask)[b, :, 0],
            np.asarray(conv_w), np.asarray(wq), np.asarray(bq), np.asarray(wk),
            np.asarray(bk), np.asarray(wv), np.asarray(bv), np.asarray(wo),
            np.asarray(bo), np.asarray(w1), np.asarray(b1), np.asarray(w2),
            np.asarray(b2), np.asarray(g1), np.asarray(beta1), np.asarray(g2),
            np.asarray(beta2)))
    res = run_bass_kernel_spmd(nc, in_maps, list(range(B)), trace=_trace)
    out = np.stack([res.results[i]["out"] for i in range(B)], axis=0)
    if _trace:
        return out, res
    return out


# revision 16
# speedup vs baseline: 1.2348x; 1.0317x over previous
"""Trainium2 Bass/Tile kernel for nn_EncoderLayer (dense transformer block).

Strategy: pure data-parallel over batch (B=8 -> 1 batch element per core, no
collectives). Per core, activations are kept feature-major ([D, T]) in bf16
(same PE matmul rate as fp32r, half the DMA/SBUF, 2x DVE). Attention folds
the key mask into the exp bias (per-partition = per-key) and appends a ones
column to V so P@V yields the softmax normalizer for free; the attention
inner loop is software-pipelined (scores lag PV by one time-tile) so the
Activation engine's exp stream never starves. h1 stays resident in SBUF
(no DRAM round trip). FFN1 output (all 40 row-tiles) stays resident in bf16,
so FFN2 accumulates entirely in PSUM feature-major (no SBUF accumulation
adds) and LayerNorm2 statistics stream on the PE during the k-sweep; the
final transpose to row-major is done with PE transpose matmuls. Partition
broadcasts for LN run on the otherwise-idle GpSimd/Pool engine.
"""

import json
import sys

if "/opt/trn_rl_repo" not in sys.path:
    sys.path.insert(0, "/opt/trn_rl_repo")

import numpy as np
import ml_dtypes

import concourse.bass as bass
import concourse.mybir as mybir
import concourse.tile as tile
from concourse import library_config

B, T, CC, DM, H, DH, DFF, K = 8, 1024, 256, 1024, 16, 64, 5120, 3
EMB = CC + DM  # 1280
EPS = 1e-6
f32 = mybir.dt.float32
bf16 = mybir.dt.bfloat16
AF = mybir.ActivationFunctionType
OP = mybir.AluOpType

NT = T // 128          # 8 time tiles
NKE = EMB // 128       # 10 embed k-tiles
NKD = DM // 128        # 8 d_model k-tiles
NMF = DFF // 128       # 40 d_ff tiles
HV = DH + 1            # 65: per-head V columns + normalizer ones column
MASK_NEG = -60000.0    # exp(-60000 + s/8) == 0.0 in f32


def _mm(nc, out, lhsT, rhs, start, stop):
    nc.tensor.matmul(out, lhsT, rhs, start=start, stop=stop)


def build_nc():
    nc = bass.Bass()

    xt_d = nc.declare_dram_parameter("xt", [EMB, T], bf16, isOutput=False)
    wv_d = nc.declare_dram_parameter("wv", [DM, DM], bf16, isOutput=False)
    wqr_d = nc.declare_dram_parameter("wqr", [8, 128, 8, 128], bf16, isOutput=False)
    wkr_d = nc.declare_dram_parameter("wkr", [8, 128, 8, 128], bf16, isOutput=False)
    wor_d = nc.declare_dram_parameter("wor", [8, 128, 8, 128], bf16, isOutput=False)
    w1r_d = nc.declare_dram_parameter("w1r", [40, 128, 10, 128], bf16, isOutput=False)
    w2r_d = nc.declare_dram_parameter("w2r", [10, 128, 40, 128], bf16, isOutput=False)
    bvf_d = nc.declare_dram_parameter("bvf", [128, DM], bf16, isOutput=False)
    mbias_d = nc.declare_dram_parameter("mbias", [128, 8], f32, isOutput=False)
    bqp_d = nc.declare_dram_parameter("bqp", [128, 8], f32, isOutput=False)
    bkp_d = nc.declare_dram_parameter("bkp", [128, 8], f32, isOutput=False)
    bop_d = nc.declare_dram_parameter("bop", [128, 8], f32, isOutput=False)
    b1p_d = nc.declare_dram_parameter("b1p", [128, 40], f32, isOutput=False)
    b2p_d = nc.declare_dram_parameter("b2p", [128, 10], f32, isOutput=False)
    g1p_d = nc.declare_dram_parameter("g1p", [128, 10], f32, isOutput=False)
    beta1p_d = nc.declare_dram_parameter("beta1p", [128, 10], f32, isOutput=False)
    g2p_d = nc.declare_dram_parameter("g2p", [128, 10], f32, isOutput=False)
    beta2p_d = nc.declare_dram_parameter("beta2p", [128, 10], f32, isOutput=False)
    cwbc_d = nc.declare_dram_parameter("cwbc", [128, K], f32, isOutput=False)
    seqrow_d = nc.declare_dram_parameter("seqrow", [1, T], f32, isOutput=False)
    onescol_d = nc.declare_dram_parameter("onescol", [128, 1], bf16, isOutput=False)
    ident_d = nc.declare_dram_parameter("ident", [128, 128], bf16, isOutput=False)
    out_d = nc.declare_dram_parameter("out", [T, EMB], f32, isOutput=True)

    with tile.TileContext(nc) as tc:
        nc.gpsimd.load_library(library_config.attnmlp)

        # ---------------- persistent pools (alloc in reverse-death order) ---
        constp = tc.alloc_tile_pool(name="constp", bufs=1)
        h1p = tc.alloc_tile_pool(name="h1p", bufs=1)
        h1 = h1p.tile([128, NKE, T], bf16)
        h1prep = tc.alloc_tile_pool(name="h1prep", bufs=1)
        h1pre = h1prep.tile([128, NKE, T], bf16)
        attp = tc.alloc_tile_pool(name="attp", bufs=1)
        attT = attp.tile([128, NKD, T], bf16)
        xtp = tc.alloc_tile_pool(name="xtp", bufs=1)
        xt = xtp.tile([128, NKE, T], bf16)

        # const tiles allocated now; DMAs deferred past the startup stream
        bvF = constp.tile([128, DM], bf16)
        mbias = constp.tile([128, 8], f32)
        bqP = constp.tile([128, 8], f32)
        bkP = constp.tile([128, 8], f32)
        boP = constp.tile([128, 8], f32)
        b1P = constp.tile([128, 40], f32)
        b2P = constp.tile([128, 10], f32)
        g1P = constp.tile([128, 10], f32)
        beta1P = constp.tile([128, 10], f32)
        g2P = constp.tile([128, 10], f32)
        beta2P = constp.tile([128, 10], f32)
        cwbc = constp.tile([128, K], f32)
        seq_row = constp.tile([1, T], f32)
        ones_col = constp.tile([128, 1], bf16)
        ident = constp.tile([128, 128], bf16)
        epsP = constp.tile([128, 1], f32)
        nc.gpsimd.memset(epsP[:], EPS)

        def emit_const_dmas():
            nc.sync.dma_start(bvF[:], bvf_d[:])
            nc.sync.dma_start(mbias[:], mbias_d[:])
            nc.sync.dma_start(bqP[:], bqp_d[:])
            nc.sync.dma_start(bkP[:], bkp_d[:])
            nc.sync.dma_start(boP[:], bop_d[:])
            nc.sync.dma_start(b1P[:], b1p_d[:])
            nc.sync.dma_start(b2P[:], b2p_d[:])
            nc.sync.dma_start(g1P[:], g1p_d[:])
            nc.sync.dma_start(beta1P[:], beta1p_d[:])
            nc.sync.dma_start(g2P[:], g2p_d[:])
            nc.sync.dma_start(beta2P[:], beta2p_d[:])
            nc.sync.dma_start(cwbc[:], cwbc_d[:])
            nc.sync.dma_start(seq_row[:], seqrow_d[:])
            nc.sync.dma_start(ones_col[:], onescol_d[:])
            nc.sync.dma_start(ident[:], ident_d[:])
            for kk in range(2):  # conv feature tiles
                nc.sync.dma_start(xt[:, kk, :], xt_d[kk * 128:(kk + 1) * 128, :])

        vp = tc.alloc_tile_pool(name="vp", bufs=1)
        vaug = vp.tile([128, NT, H * HV], bf16)
        # normalizer ones column (col DH of each head slot)
        ocols = vaug.rearrange("p j (h c) -> p (j h) c", c=HV)[:, :, DH:HV]
        nc.gpsimd.memset(ocols, 1.0)

        qkp = tc.alloc_tile_pool(name="qkp", bufs=1)
        qt = qkp.tile([128, NKD, T], bf16)
        kt = qkp.tile([128, NKD, T], bf16)

        with tc.tile_pool(name="wqp", bufs=3) as wqp:
            qk_wts = []

            def emit_qk_load(i):
                wdram = wqr_d if i < 8 else wkr_d
                wt = wqp.tile([128, 8, 128], bf16, tag="wt")
                nc.sync.dma_start(wt[:], wdram[i % 8])
                qk_wts.append(wt)

            # ------------ V projection (row-major, bias, augmented) --------
            with (
                tc.tile_pool(name="wvp", bufs=3) as wvp,
                tc.tile_pool(name="vps", bufs=8, space="PSUM") as vps,
            ):
                for n in range(2):
                    pss = [vps.tile([128, 512], f32, name=f"vps{i}", tag="vps")
                           for i in range(NT)]
                    for k in range(NKD):
                        if n == 0:  # startup: interleave x and wv streams
                            nc.sync.dma_start(
                                xt[:, 2 + k, :],
                                xt_d[(2 + k) * 128:(3 + k) * 128, :])
                        wvt = wvp.tile([128, 512], bf16)
                        nc.sync.dma_start(
                            wvt[:],
                            wv_d[k * 128:(k + 1) * 128, n * 512:(n + 1) * 512])
                        for i in range(NT):
                            _mm(nc, pss[i][:], xt[:, 2 + k, i * 128:(i + 1) * 128],
                                wvt[:], k == 0, k == NKD - 1)
                    if n == 0:
                        emit_const_dmas()
                        emit_qk_load(0)
                        emit_qk_load(1)
                    for i in range(NT):
                        dest = vaug[:, i, :].rearrange("p (h c) -> p h c", c=HV)
                        dest = dest[:, n * 8:(n + 1) * 8, 0:DH]
                        nc.vector.tensor_add(dest, pss[i][:],
                                             bvF[:, n * 512:(n + 1) * 512])

            # ------------ Q/K projections (feature-major) ------------------
            with tc.tile_pool(name="qps", bufs=2, space="PSUM") as qps:
                for i in range(16):
                    if i + 2 < 16:
                        emit_qk_load(i + 2)
                    m = i % 8
                    dst, biasP = (qt, bqP) if i < 8 else (kt, bkP)
                    wt = qk_wts[i]
                    ps = qps.tile([128, 2, 512], f32)
                    for n in range(2):
                        for k in range(NKD):
                            _mm(nc, ps[:, n, :], wt[:, k, :],
                                xt[:, 2 + k, n * 512:(n + 1) * 512],
                                k == 0, k == NKD - 1)
                    nc.vector.tensor_scalar_add(
                        dst[:, m, :], ps.rearrange("p a b -> p (a b)"),
                        biasP[:, m:m + 1])

        # ---------------- attention (single head, lag-1 pipelined) ---------
        with (
            tc.tile_pool(name="upool", bufs=3) as upool,
            tc.tile_pool(name="normp", bufs=2) as normp,
            tc.tile_pool(name="bcp", bufs=2) as bcp,
            tc.tile_pool(name="sps", bufs=2, space="PSUM") as spsp,
            tc.tile_pool(name="aps", bufs=2, space="PSUM") as apsp,
        ):
            def scores_exp(h, jt):
                prow = (h % 2) * 64
                ktile = h // 2
                sps = spsp.tile([128, 2, 512], f32, name="sps", tag="sps")
                klhs = kt[prow:prow + 64, ktile, jt * 128:(jt + 1) * 128]
                for c in range(2):
                    _mm(nc, sps[:, c, :], klhs,
                        qt[prow:prow + 64, ktile, c * 512:(c + 1) * 512],
                        True, True)
                u = upool.tile([128, T], bf16, name="u", tag="u")
                nc.scalar.activation(
                    u[:], sps.rearrange("p a b -> p (a b)"), AF.Exp,
                    scale=0.125, bias=mbias[:, jt:jt + 1])
                return u

            def pv(h, jt, u, aps):
                vlhs = vaug[:, jt, h * HV:(h + 1) * HV]
                for c in range(2):
                    _mm(nc, aps[:, c, :], vlhs,
                        u[:, c * 512:(c + 1) * 512], jt == 0, jt == NT - 1)

            def evacuate(h, aps):
                # 1/normalizer; unnormalized attention rows -> attT
                prow = (h % 2) * 64
                ktile = h // 2
                nt_ = normp.tile([1, T], bf16, name="nt", tag="nt")
                with nc.allow_low_precision(reason="bf16 softmax normalizer"):
                    nc.vector.reciprocal(
                        nt_[:], aps[DH:HV, :, :].rearrange("p a b -> p (a b)"))
                nc.vector.tensor_copy(
                    attT[prow:prow + 64, ktile, :],
                    aps[0:DH, :, :].rearrange("p a b -> p (a b)"))
                return nt_

            def finalize(h, nt_):
                prow = (h % 2) * 64
                ktile = h // 2
                bc = bcp.tile([64, T], bf16, name="bc", tag="bc")
                nc.gpsimd.partition_broadcast(bc[:], nt_[0:1, :])
                nc.vector.tensor_mul(
                    attT[prow:prow + 64, ktile, :],
                    attT[prow:prow + 64, ktile, :], bc[:])

            pending = None  # (h, norm_tile) awaiting broadcast+scale
            for h in range(H):
                aps = apsp.tile([HV, 2, 512], f32, name="aps", tag="aps")
                us = [scores_exp(h, 0), scores_exp(h, 1)]
                for jt in range(NT):
                    if jt + 2 < NT:
                        us.append(scores_exp(h, jt + 2))
                    pv(h, jt, us[jt], aps)
                nt_ = evacuate(h, aps)
                if pending is not None:
                    finalize(*pending)
                pending = (h, nt_)
            finalize(*pending)

        qkp.release()
        vp.release()

        # ---------------- h1pre = concat(conv, att@wo + bo) + x ------------
        with (
            tc.tile_pool(name="convp", bufs=2) as convp,
            tc.tile_pool(name="wop", bufs=3) as wop,
            tc.tile_pool(name="ops", bufs=4, space="PSUM") as opsp,
            tc.tile_pool(name="lnps", bufs=1, space="PSUM") as lnps,
            tc.tile_pool(name="sqp", bufs=3) as sqp,
            tc.tile_pool(name="vecp", bufs=1) as vecp,
        ):
            musum = lnps.tile([1, 2, 512], f32, tag="musum")
            sqsum = lnps.tile([1, 2, 512], f32, tag="sqsum")

            def ln1_k(kb):
                sq = sqp.tile([128, T], bf16, tag="sq")
                nc.vector.tensor_mul(sq[:], h1pre[:, kb, :], h1pre[:, kb, :])
                for c in range(2):
                    _mm(nc, musum[:, c, :], ones_col[:],
                        h1pre[:, kb, c * 512:(c + 1) * 512], kb == 0, kb == NKE - 1)
                    _mm(nc, sqsum[:, c, :], ones_col[:],
                        sq[:, c * 512:(c + 1) * 512], kb == 0, kb == NKE - 1)

            # depthwise conv (DVE) on the first two feature tiles
            for kb in range(2):
                pad = convp.tile([128, T + 2], bf16, tag="pad")
                nc.gpsimd.memset(pad[:, 0:1], 0.0)
                nc.gpsimd.memset(pad[:, T + 1:T + 2], 0.0)
                nc.vector.tensor_copy(pad[:, 1:T + 1], xt[:, kb, :])
                a1 = convp.tile([128, T], bf16, tag="a1")
                nc.vector.tensor_scalar_mul(a1[:], pad[:, 0:T], cwbc[:, 0:1])
                a2 = convp.tile([128, T], bf16, tag="a2")
                nc.vector.scalar_tensor_tensor(
                    a2[:], pad[:, 1:T + 1], cwbc[:, 1:2], a1[:], OP.mult, OP.add)
                a3 = convp.tile([128, T], bf16, tag="a3")
                nc.vector.scalar_tensor_tensor(
                    a3[:], pad[:, 2:T + 2], cwbc[:, 2:3], a2[:], OP.mult, OP.add)
                nc.vector.tensor_add(h1pre[:, kb, :], a3[:], xt[:, kb, :])
                ln1_k(kb)

            # attention out-projection with residual seeded via identity
            for m in range(8):
                wt = wop.tile([128, 8, 128], bf16, tag="wo")
                nc.sync.dma_start(wt[:], wor_d[m])
                for n in range(2):
                    ps = opsp.tile([128, 512], f32)
                    _mm(nc, ps[:], ident[:], xt[:, 2 + m, n * 512:(n + 1) * 512],
                        True, False)
                    for k in range(NKD):
                        _mm(nc, ps[:], wt[:, k, :],
                            attT[:, k, n * 512:(n + 1) * 512], False, k == NKD - 1)
                    nc.scalar.activation(
                        h1pre[:, 2 + m, n * 512:(n + 1) * 512], ps[:], AF.Identity,
                        bias=boP[:, m:m + 1])
                ln1_k(2 + m)

            # LayerNorm 1 statistics + normalize (feature axis = partitions).
            # Chain split across DVE/Act/Pool to shorten the critical path.
            mu = vecp.tile([1, T], f32, tag="mu")
            nc.vector.tensor_scalar_mul(
                mu[:], musum.rearrange("p a b -> p (a b)"), 1.0 / EMB)
            mu2 = vecp.tile([1, T], f32, tag="mu2s")
            nc.scalar.activation(mu2[:], mu[:], AF.Square)
            ex2 = vecp.tile([1, T], f32, tag="ex2")
            nc.vector.tensor_scalar(
                ex2[:], sqsum.rearrange("p a b -> p (a b)"), 1.0 / EMB, EPS,
                OP.mult, OP.add)
            muB = vecp.tile([1, T], bf16, tag="muB")
            with nc.allow_low_precision(reason="bf16 LN1 factors"):
                nc.gpsimd.tensor_copy(muB[:], mu[:])
            muF = vecp.tile([128, T], bf16, tag="muF")
            nc.gpsimd.partition_broadcast(muF[:], muB[0:1, :])
            var = vecp.tile([1, T], f32, tag="var")
            nc.vector.tensor_sub(var[:], ex2[:], mu2[:])  # includes +eps
            vrec = vecp.tile([1, T], f32, tag="vrec")
            nc.vector.reciprocal(vrec[:], var[:])
            rs = vecp.tile([1, T], f32, tag="rs")
            nc.scalar.activation(rs[:], vrec[:], AF.Sqrt)
            rsB = vecp.tile([1, T], bf16, tag="rsB")
            with nc.allow_low_precision(reason="bf16 LN1 factors"):
                nc.vector.tensor_mul(rsB[:], rs[:], seq_row[:])  # fold seq_mask
            rsF = vecp.tile([128, T], bf16, tag="rsF")
            nc.gpsimd.partition_broadcast(rsF[:], rsB[0:1, :])
            for kb in range(NKE):
                t1 = sqp.tile([128, T], bf16, tag="t1")
                nc.vector.tensor_sub(t1[:], h1pre[:, kb, :], muF[:])
                t2 = sqp.tile([128, T], bf16, tag="t2")
                nc.vector.tensor_mul(t2[:], t1[:], rsF[:])
                nc.scalar.activation(
                    h1[:, kb, :], t2[:], AF.Identity,
                    bias=beta1P[:, kb:kb + 1], scale=g1P[:, kb:kb + 1])

        xtp.release()
        attp.release()
        h1prep.release()

        # ---------------- FFN1: ffb[m] = relu(h1 @ w1 + b1), all resident --
        outp = tc.alloc_tile_pool(name="outp", bufs=1)
        oacc = outp.tile([128, NKE, T], bf16)
        ffbp = tc.alloc_tile_pool(name="ffbp", bufs=1)
        ffb = ffbp.tile([128, NMF, T], bf16)
        w2ctx = tc.tile_pool(name="w2p", bufs=2)
        w2p = w2ctx.__enter__()
        w2ts = {}

        def load_w2(e):
            t = w2p.tile([128, 40, 128], bf16, tag="w2t")
            nc.sync.dma_start(t[:], w2r_d[e])
            w2ts[e] = t

        with (
            tc.tile_pool(name="w1p", bufs=3) as w1p,
            tc.tile_pool(name="ps1", bufs=3, space="PSUM") as ps1,
        ):
            for mf in range(NMF):
                w1t = w1p.tile([128, 10, 128], bf16, tag="w1t")
                nc.sync.dma_start(w1t[:], w1r_d[mf])
                if mf == 6:
                    load_w2(0)
                if mf == 24:
                    load_w2(1)
                ps = ps1.tile([128, 2, 512], f32)
                for k in range(NKE):
                    for c in range(2):
                        _mm(nc, ps[:, c, :], w1t[:, k, :],
                            h1[:, k, c * 512:(c + 1) * 512], k == 0, k == NKE - 1)
                nc.scalar.activation(
                    ffb[:, mf, :], ps.rearrange("p a b -> p (a b)"),
                    AF.Relu, bias=b1P[:, mf:mf + 1])

        # ---------------- FFN2 + LayerNorm 2 (feature-major) ---------------
        with (
            tc.tile_pool(name="ps2", bufs=2, space="PSUM") as ps2,
            tc.tile_pool(name="lnps2", bufs=1, space="PSUM") as lnps2,
            tc.tile_pool(name="sq2p", bufs=3) as sq2p,
            tc.tile_pool(name="vec2p", bufs=1) as vec2p,
        ):
            musum2 = lnps2.tile([1, 2, 512], f32, tag="musum2")
            sqsum2 = lnps2.tile([1, 2, 512], f32, tag="sqsum2")

            def emit_stats2(es, sqs):
                for c in range(2):
                    _mm(nc, musum2[:, c, :], ones_col[:],
                        oacc[:, es, c * 512:(c + 1) * 512], es == 0, es == NKE - 1)
                    _mm(nc, sqsum2[:, c, :], ones_col[:],
                        sqs[:, c * 512:(c + 1) * 512], es == 0, es == NKE - 1)

            pending_stats = None
            for e in range(NKE):
                if e + 1 < NKE and e + 1 not in w2ts:
                    load_w2(e + 1)
                w2t = w2ts.pop(e)
                pso = ps2.tile([128, 2, 512], f32)
                for k in range(NMF):
                    for c in range(2):
                        _mm(nc, pso[:, c, :], w2t[:, k, :],
                            ffb[:, k, c * 512:(c + 1) * 512], k == 0, k == NMF - 1)
                    if k == 8 and pending_stats is not None:
                        # stats for e-1 land mid-sweep so the PE never waits
                        # on the DVE epilogue of tile e-1
                        emit_stats2(*pending_stats)
                        pending_stats = None
                # oacc[e] = (pso + b2) + h1[e]   (residual2)
                nc.vector.scalar_tensor_tensor(
                    oacc[:, e, :], pso.rearrange("p a b -> p (a b)"),
                    b2P[:, e:e + 1], h1[:, e, :], OP.add, OP.add)
                sq = sq2p.tile([128, T], bf16, tag="sq2")
                nc.vector.tensor_mul(sq[:], oacc[:, e, :], oacc[:, e, :])
                pending_stats = (e, sq)
            emit_stats2(*pending_stats)

            mu = vec2p.tile([1, T], f32, tag="mu2")
            nc.vector.tensor_scalar_mul(
                mu[:], musum2.rearrange("p a b -> p (a b)"), 1.0 / EMB)
            mu2 = vec2p.tile([1, T], f32, tag="mu2s")
            nc.scalar.activation(mu2[:], mu[:], AF.Square)
            ex2 = vec2p.tile([1, T], f32, tag="ex22")
            nc.vector.tensor_scalar(
                ex2[:], sqsum2.rearrange("p a b -> p (a b)"), 1.0 / EMB, EPS,
                OP.mult, OP.add)
            muB = vec2p.tile([1, T], bf16, tag="muB2")
            with nc.allow_low_precision(reason="bf16 LN2 factors"):
                nc.gpsimd.tensor_copy(muB[:], mu[:])
            muF = vec2p.tile([128, T], bf16, tag="muF2")
            nc.gpsimd.partition_broadcast(muF[:], muB[0:1, :])
            var = vec2p.tile([1, T], f32, tag="var2")
            nc.vector.tensor_sub(var[:], ex2[:], mu2[:])  # includes +eps
            vrec = vec2p.tile([1, T], f32, tag="vrec2")
            nc.vector.reciprocal(vrec[:], var[:])
            rs = vec2p.tile([1, T], f32, tag="rs2")
            nc.scalar.activation(rs[:], vrec[:], AF.Sqrt)
            rsB = vec2p.tile([1, T], bf16, tag="rsB2")
            with nc.allow_low_precision(reason="bf16 LN2 factors"):
                nc.vector.tensor_mul(rsB[:], rs[:], seq_row[:])
            rsF = vec2p.tile([128, T], bf16, tag="rsF2")
            nc.gpsimd.partition_broadcast(rsF[:], rsB[0:1, :])
            for e in range(NKE):
                t1 = sq2p.tile([128, T], bf16, tag="t12")
                nc.vector.tensor_sub(t1[:], oacc[:, e, :], muF[:])
                t2 = sq2p.tile([128, T], bf16, tag="t22")
                nc.vector.tensor_mul(t2[:], t1[:], rsF[:])
                nc.scalar.activation(
                    oacc[:, e, :], t2[:], AF.Identity,
                    bias=beta2P[:, e:e + 1], scale=g2P[:, e:e + 1])

        # ---------------- transpose to row-major + store --------------------
        w2ctx.__exit__(None, None, None)
        ffbp.release()
        with (
            tc.tile_pool(name="psT", bufs=2, space="PSUM") as psT,
            tc.tile_pool(name="obuf", bufs=3) as obuf,
        ):
            for tb in range(NT):
                pt = psT.tile([128, NKE, 128], bf16)
                for e in range(NKE):
                    nc.tensor.matmul(
                        pt[:, e, :], oacc[:, e, tb * 128:(tb + 1) * 128],
                        ident[:], start=True, stop=True, is_transpose=True)
                ob = obuf.tile([128, EMB], f32)
                if tb % 2 == 0:
                    nc.scalar.activation(
                        ob[:], pt.rearrange("p a b -> p (a b)"), AF.Identity)
                else:
                    nc.vector.tensor_copy(ob[:], pt.rearrange("p a b -> p (a b)"))
                nc.sync.dma_start(out_d[tb * 128:(tb + 1) * 128, :], ob[:])

        outp.release()
        h1p.release()
        constp.release()

    return nc


def _split_matmul_waits(bj: bytes) -> bytes:
    """Walrus codegen allows only one sync-wait on Matmult/DMACopy
    instructions; hoist extra waits onto a preceding EventSemaphore."""
    d = json.loads(bj)
    n = 0
    for f in d["functions"]:
        for blk in f["blocks"]:
            out = []
            for inst in blk["instructions"]:
                si = inst.get("sync_info")
                if (si and si.get("on_wait") and len(si["on_wait"]) >= 2
                        and inst.get("opcode") != "EventSemaphore"):
                    waits = si["on_wait"]
                    for w in waits[:-1]:
                        out.append({
                            "debug": inst.get("debug"),
                            "engine": inst["engine"],
                            "ins": [],
                            "outs": [],
                            "name": f"waitfix_{n}",
                            "opcode": "EventSemaphore",
                            "sync_info": {"on_update": [], "on_wait": [w]},
                        })
                        n += 1
                    si["on_wait"] = waits[-1:]
                out.append(inst)
            blk["instructions"] = out
    return json.dumps(d).encode()


_NC_CACHE = None


def _get_nc():
    global _NC_CACHE
    if _NC_CACHE is None:
        nc = build_nc()
        orig = nc.to_json_bytes
        nc.to_json_bytes = lambda: _split_matmul_waits(orig())
        _NC_CACHE = nc
    return _NC_CACHE


def _prep_core_inputs(x_b, mask_b, seq_b, conv_w, wq, bq, wk, bk, wv, bv, wo, bo,
                      w1, b1, w2, b2, g1, beta1, g2, beta2):
    f = np.float32
    bf = ml_dtypes.bfloat16
    mask_b = np.asarray(mask_b)
    masked = (mask_b != 0).astype(f)  # reference: att_mask != 0 -> -1e9 score
    return {
        "xt": np.ascontiguousarray(x_b.T).astype(bf),
        "wv": np.ascontiguousarray(wv).astype(bf),
        "wqr": np.ascontiguousarray(
            wq.reshape(8, 128, 8, 128).transpose(2, 1, 0, 3)).astype(bf),
        "wkr": np.ascontiguousarray(
            wk.reshape(8, 128, 8, 128).transpose(2, 1, 0, 3)).astype(bf),
        "wor": np.ascontiguousarray(
            wo.reshape(8, 128, 8, 128).transpose(2, 1, 0, 3)).astype(bf),
        "w1r": np.ascontiguousarray(
            w1.reshape(10, 128, 40, 128).transpose(2, 1, 0, 3)).astype(bf),
        "w2r": np.ascontiguousarray(
            w2.reshape(40, 128, 10, 128).transpose(2, 1, 0, 3)).astype(bf),
        "bvf": np.tile(np.asarray(bv, f)[None, :], (128, 1)).astype(bf),
        "mbias": np.ascontiguousarray(
            (MASK_NEG * masked).reshape(8, 128).T.astype(f)),
        "bqp": np.ascontiguousarray(np.asarray(bq, f).reshape(8, 128).T),
        "bkp": np.ascontiguousarray(np.asarray(bk, f).reshape(8, 128).T),
        "bop": np.ascontiguousarray(np.asarray(bo, f).reshape(8, 128).T),
        "b1p": np.ascontiguousarray(np.asarray(b1, f).reshape(40, 128).T),
        "b2p": np.ascontiguousarray(np.asarray(b2, f).reshape(10, 128).T),
        "g1p": np.ascontiguousarray(np.asarray(g1, f).reshape(10, 128).T),
        "beta1p": np.ascontiguousarray(np.asarray(beta1, f).reshape(10, 128).T),
        "g2p": np.ascontiguousarray(np.asarray(g2, f).reshape(10, 128).T),
        "beta2p": np.ascontiguousarray(np.asarray(beta2, f).reshape(10, 128).T),
        "cwbc": np.tile(np.asarray(conv_w, f).reshape(K)[None, :], (128, 1)),
        "seqrow": np.ascontiguousarray(np.asarray(seq_b, f).reshape(1, T)),
        "onescol": np.ones((128, 1), bf),
        "ident": np.eye(128, dtype=f).astype(bf),
    }


def kernel(x, att_mask, seq_mask, conv_w, wq, bq, wk, bk, wv, bv, wo, bo,
           w1, b1, w2, b2, g1, beta1, g2, beta2, _trace=False):
    from concourse.bass_utils import run_bass_kernel_spmd

    nc = _get_nc()
    x = np.asarray(x, dtype=np.float32)
    in_maps = []
    for b in range(B):
        in_maps.append(_prep_core_inputs(
            x[b], np.asarray(att_mask)[b], np.asarray(seq_mask)[b, :, 0],
            np.asarray(conv_w), np.asarray(wq), np.asarray(bq), np.asarray(wk),
            np.asarray(bk), np.asarray(wv), np.asarray(bv), np.asarray(wo),
            np.asarray(bo), np.asarray(w1), np.asarray(b1), np.asarray(w2),
            np.asarray(b2), np.asarray(g1), np.asarray(beta1), np.asarray(g2),
            np.asarray(beta2)))
    res = run_bass_kernel_spmd(nc, in_maps, list(range(B)), trace=_trace)
    out = np.stack([res.results[i]["out"] for i in range(B)], axis=0)
    if _trace:
        return out, res
    return out


# revision 21
# speedup vs baseline: 1.2691x; 1.0277x over previous
"""Trainium2 Bass/Tile kernel for nn_EncoderLayer (dense transformer block).

Strategy: pure data-parallel over batch (B=8 -> 1 batch element per core, no
collectives). Per core, activations are kept feature-major ([D, T]) in bf16
(same PE matmul rate as fp32r, half the DMA/SBUF, 2x DVE). Attention folds
the key mask into the exp bias (per-partition = per-key) and appends a ones
column to V so P@V yields the softmax normalizer for free; the attention
inner loop is software-pipelined (scores lag PV by one time-tile) so the
Activation engine's exp stream never starves. h1 stays resident in SBUF
(no DRAM round trip). FFN1 output (all 40 row-tiles) stays resident in bf16,
so FFN2 accumulates entirely in PSUM feature-major (no SBUF accumulation
adds) and LayerNorm2 statistics stream on the PE during the k-sweep; the
final transpose to row-major is done with PE transpose matmuls. Partition
broadcasts for LN run on the otherwise-idle GpSimd/Pool engine.
"""

import json
import sys

if "/opt/trn_rl_repo" not in sys.path:
    sys.path.insert(0, "/opt/trn_rl_repo")

import numpy as np
import ml_dtypes

import concourse.bass as bass
import concourse.mybir as mybir
import concourse.tile as tile
from concourse import library_config

B, T, CC, DM, H, DH, DFF, K = 8, 1024, 256, 1024, 16, 64, 5120, 3
EMB = CC + DM  # 1280
EPS = 1e-6
f32 = mybir.dt.float32
bf16 = mybir.dt.bfloat16
AF = mybir.ActivationFunctionType
OP = mybir.AluOpType

NT = T // 128          # 8 time tiles
NKE = EMB // 128       # 10 embed k-tiles
NKD = DM // 128        # 8 d_model k-tiles
NMF = DFF // 128       # 40 d_ff tiles
HV = DH + 1            # 65: per-head V columns + normalizer ones column
MASK_NEG = -60000.0    # exp(-60000 + s/8) == 0.0 in f32


def _mm(nc, out, lhsT, rhs, start, stop):
    nc.tensor.matmul(out, lhsT, rhs, start=start, stop=stop)


def build_nc():
    nc = bass.Bass()

    xt_d = nc.declare_dram_parameter("xt", [EMB, T], bf16, isOutput=False)
    wv_d = nc.declare_dram_parameter("wv", [DM, DM], bf16, isOutput=False)
    wqr_d = nc.declare_dram_parameter("wqr", [8, 128, 8, 128], bf16, isOutput=False)
    wkr_d = nc.declare_dram_parameter("wkr", [8, 128, 8, 128], bf16, isOutput=False)
    wor_d = nc.declare_dram_parameter("wor", [8, 128, 8, 128], bf16, isOutput=False)
    w1r_d = nc.declare_dram_parameter("w1r", [40, 128, 10, 128], bf16, isOutput=False)
    w2r_d = nc.declare_dram_parameter("w2r", [10, 128, 40, 128], bf16, isOutput=False)
    bvf_d = nc.declare_dram_parameter("bvf", [128, DM], bf16, isOutput=False)
    mbias_d = nc.declare_dram_parameter("mbias", [128, 8], f32, isOutput=False)
    bqp_d = nc.declare_dram_parameter("bqp", [128, 8], f32, isOutput=False)
    bkp_d = nc.declare_dram_parameter("bkp", [128, 8], f32, isOutput=False)
    bop_d = nc.declare_dram_parameter("bop", [128, 8], f32, isOutput=False)
    b1p_d = nc.declare_dram_parameter("b1p", [128, 40], f32, isOutput=False)
    b2p_d = nc.declare_dram_parameter("b2p", [128, 10], f32, isOutput=False)
    g1p_d = nc.declare_dram_parameter("g1p", [128, 10], f32, isOutput=False)
    beta1p_d = nc.declare_dram_parameter("beta1p", [128, 10], f32, isOutput=False)
    g2p_d = nc.declare_dram_parameter("g2p", [128, 10], f32, isOutput=False)
    beta2p_d = nc.declare_dram_parameter("beta2p", [128, 10], f32, isOutput=False)
    cwbc_d = nc.declare_dram_parameter("cwbc", [128, K], f32, isOutput=False)
    seqrow_d = nc.declare_dram_parameter("seqrow", [1, T], f32, isOutput=False)
    onescol_d = nc.declare_dram_parameter("onescol", [128, 1], bf16, isOutput=False)
    ident_d = nc.declare_dram_parameter("ident", [128, 128], bf16, isOutput=False)
    out_d = nc.declare_dram_parameter("out", [T, EMB], f32, isOutput=True)

    with tile.TileContext(nc) as tc:
        nc.gpsimd.load_library(library_config.proxy)

        # ---------------- persistent pools (alloc in reverse-death order) ---
        constp = tc.alloc_tile_pool(name="constp", bufs=1)
        h1p = tc.alloc_tile_pool(name="h1p", bufs=1)
        h1 = h1p.tile([128, NKE, T], bf16)
        h1prep = tc.alloc_tile_pool(name="h1prep", bufs=1)
        h1pre = h1prep.tile([128, NKE, T], bf16)
        attp = tc.alloc_tile_pool(name="attp", bufs=1)
        attT = attp.tile([128, NKD, T], bf16)
        xtp = tc.alloc_tile_pool(name="xtp", bufs=1)
        xt = xtp.tile([128, NKE, T], bf16)

        # const tiles allocated now; DMAs deferred past the startup stream
        bvF = constp.tile([128, DM], bf16)
        mbias = constp.tile([128, 8], f32)
        bqP = constp.tile([128, 8], f32)
        bkP = constp.tile([128, 8], f32)
        boP = constp.tile([128, 8], f32)
        b1P = constp.tile([128, 40], f32)
        b2P = constp.tile([128, 10], f32)
        g1P = constp.tile([128, 10], f32)
        beta1P = constp.tile([128, 10], f32)
        g2P = constp.tile([128, 10], f32)
        beta2P = constp.tile([128, 10], f32)
        cwbc = constp.tile([128, K], f32)
        seq_row = constp.tile([1, T], f32)
        ones_col = constp.tile([128, 1], bf16)
        ident = constp.tile([128, 128], bf16)
        epsP = constp.tile([128, 1], f32)
        nc.gpsimd.memset(epsP[:], EPS)
        dumt = constp.tile([1, 2], f32)
        NSTAGE = 6
        w1stage = [constp.tile([128, 10, 128], bf16, name=f"w1s{i}")
                   for i in range(NSTAGE)]

        def emit_const_dmas():
            nc.sync.dma_start(mbias[:], mbias_d[:])
            nc.sync.dma_start(bqP[:], bqp_d[:])
            nc.sync.dma_start(bkP[:], bkp_d[:])
            nc.sync.dma_start(boP[:], bop_d[:])
            nc.sync.dma_start(b1P[:], b1p_d[:])
            nc.sync.dma_start(b2P[:], b2p_d[:])
            nc.sync.dma_start(g1P[:], g1p_d[:])
            nc.sync.dma_start(beta1P[:], beta1p_d[:])
            nc.sync.dma_start(g2P[:], g2p_d[:])
            nc.sync.dma_start(beta2P[:], beta2p_d[:])
            nc.sync.dma_start(cwbc[:], cwbc_d[:])
            nc.sync.dma_start(seq_row[:], seqrow_d[:])
            nc.sync.dma_start(ones_col[:], onescol_d[:])
            nc.sync.dma_start(ident[:], ident_d[:])
            for kk in range(2):  # conv feature tiles
                nc.sync.dma_start(xt[:, kk, :], xt_d[kk * 128:(kk + 1) * 128, :])

        vp = tc.alloc_tile_pool(name="vp", bufs=1)
        vaug = vp.tile([128, NT, H * HV], bf16)
        # normalizer ones column (col DH of each head slot)
        ocols = vaug.rearrange("p j (h c) -> p (j h) c", c=HV)[:, :, DH:HV]
        nc.gpsimd.memset(ocols, 1.0)

        qkp = tc.alloc_tile_pool(name="qkp", bufs=1)
        qt = qkp.tile([128, NKD, T], bf16)
        kt = qkp.tile([128, NKD, T], bf16)

        with tc.tile_pool(name="wqp", bufs=3) as wqp:
            qk_wts = []

            def emit_qk_load(i):
                wdram = wqr_d if i < 8 else wkr_d
                wt = wqp.tile([128, 8, 128], bf16, tag="wt")
                nc.sync.dma_start(wt[:], wdram[i % 8])
                qk_wts.append(wt)

            # ------------ V projection (row-major, bias, augmented) --------
            with (
                tc.tile_pool(name="wvp", bufs=3) as wvp,
                tc.tile_pool(name="vps", bufs=8, space="PSUM") as vps,
            ):
                for n in range(2):
                    pss = [vps.tile([128, 512], f32, name=f"vps{i}", tag="vps")
                           for i in range(NT)]
                    for k in range(NKD):
                        if n == 0:  # startup: interleave x and wv streams
                            nc.sync.dma_start(
                                xt[:, 2 + k, :],
                                xt_d[(2 + k) * 128:(3 + k) * 128, :])
                        if n == 1 and k == 2:
                            emit_const_dmas()
                        if n == 1 and k == 5:
                            emit_qk_load(0)
                            emit_qk_load(1)
                        wvt = wvp.tile([128, 512], bf16)
                        nc.sync.dma_start(
                            wvt[:],
                            wv_d[k * 128:(k + 1) * 128, n * 512:(n + 1) * 512])
                        for i in range(NT):
                            _mm(nc, pss[i][:], xt[:, 2 + k, i * 128:(i + 1) * 128],
                                wvt[:], k == 0, k == NKD - 1)
                    if n == 0:
                        nc.sync.dma_start(bvF[:], bvf_d[:])
                        # preload the Exp activation table before attention
                        nc.scalar.activation(dumt[0:1, 0:1], epsP[0:1, :], AF.Exp)
                    for i in range(NT):
                        dest = vaug[:, i, :].rearrange("p (h c) -> p h c", c=HV)
                        dest = dest[:, n * 8:(n + 1) * 8, 0:DH]
                        eng = nc.vector if i % 8 < 5 else nc.gpsimd
                        eng.tensor_add(dest, pss[i][:],
                                       bvF[:, n * 512:(n + 1) * 512])

            # ------------ Q/K projections (feature-major) ------------------
            with tc.tile_pool(name="qps", bufs=2, space="PSUM") as qps:
                for i in range(16):
                    if i + 2 < 16:
                        emit_qk_load(i + 2)
                    m = i % 8
                    dst, biasP = (qt, bqP) if i < 8 else (kt, bkP)
                    wt = qk_wts[i]
                    ps = qps.tile([128, 2, 512], f32)
                    for n in range(2):
                        for k in range(NKD):
                            _mm(nc, ps[:, n, :], wt[:, k, :],
                                xt[:, 2 + k, n * 512:(n + 1) * 512],
                                k == 0, k == NKD - 1)
                    nc.vector.tensor_scalar_add(
                        dst[:, m, :], ps.rearrange("p a b -> p (a b)"),
                        biasP[:, m:m + 1])

        # ---------------- attention (single head, lag-1 pipelined) ---------
        with (
            tc.tile_pool(name="upool", bufs=3) as upool,
            tc.tile_pool(name="normp", bufs=2) as normp,
            tc.tile_pool(name="bcp", bufs=2) as bcp,
            tc.tile_pool(name="sps", bufs=2, space="PSUM") as spsp,
            tc.tile_pool(name="aps", bufs=2, space="PSUM") as apsp,
        ):
            def scores_exp(h, jt):
                prow = (h % 2) * 64
                ktile = h // 2
                sps = spsp.tile([128, 2, 512], f32, name="sps", tag="sps")
                klhs = kt[prow:prow + 64, ktile, jt * 128:(jt + 1) * 128]
                for c in range(2):
                    _mm(nc, sps[:, c, :], klhs,
                        qt[prow:prow + 64, ktile, c * 512:(c + 1) * 512],
                        True, True)
                u = upool.tile([128, T], bf16, name="u", tag="u")
                nc.scalar.activation(
                    u[:], sps.rearrange("p a b -> p (a b)"), AF.Exp,
                    scale=0.125, bias=mbias[:, jt:jt + 1])
                return u

            def pv(h, jt, u, aps):
                vlhs = vaug[:, jt, h * HV:(h + 1) * HV]
                for c in range(2):
                    _mm(nc, aps[:, c, :], vlhs,
                        u[:, c * 512:(c + 1) * 512], jt == 0, jt == NT - 1)

            def evacuate(h, aps):
                # 1/normalizer; unnormalized attention rows -> attT
                prow = (h % 2) * 64
                ktile = h // 2
                nt_ = normp.tile([1, T], bf16, name="nt", tag="nt")
                with nc.allow_low_precision(reason="bf16 softmax normalizer"):
                    nc.vector.reciprocal(
                        nt_[:], aps[DH:HV, :, :].rearrange("p a b -> p (a b)"))
                nc.vector.tensor_copy(
                    attT[prow:prow + 64, ktile, :],
                    aps[0:DH, :, :].rearrange("p a b -> p (a b)"))
                return nt_

            def finalize(h, nt_):
                prow = (h % 2) * 64
                ktile = h // 2
                bc = bcp.tile([64, T], bf16, name="bc", tag="bc")
                nc.gpsimd.partition_broadcast(bc[:], nt_[0:1, :])
                nc.vector.tensor_mul(
                    attT[prow:prow + 64, ktile, :],
                    attT[prow:prow + 64, ktile, :], bc[:])

            pending = None  # (h, norm_tile) awaiting broadcast+scale
            for h in range(H):
                aps = apsp.tile([HV, 2, 512], f32, name="aps", tag="aps")
                us = [scores_exp(h, 0), scores_exp(h, 1)]
                for jt in range(NT):
                    if jt + 2 < NT:
                        us.append(scores_exp(h, jt + 2))
                    pv(h, jt, us[jt], aps)
                nt_ = evacuate(h, aps)
                if pending is not None:
                    finalize(*pending)
                pending = (h, nt_)
            finalize(*pending)

        qkp.release()
        vp.release()

        # ---------------- h1pre = concat(conv, att@wo + bo) + x ------------
        with (
            tc.tile_pool(name="convp", bufs=2) as convp,
            tc.tile_pool(name="wop", bufs=3) as wop,
            tc.tile_pool(name="ops", bufs=4, space="PSUM") as opsp,
            tc.tile_pool(name="lnps", bufs=1, space="PSUM") as lnps,
            tc.tile_pool(name="sqp", bufs=3) as sqp,
            tc.tile_pool(name="vecp", bufs=1) as vecp,
        ):
            musum = lnps.tile([1, 2, 512], f32, tag="musum")
            sqsum = lnps.tile([1, 2, 512], f32, tag="sqsum")

            def ln1_k(kb):
                sq = sqp.tile([128, T], bf16, tag="sq")
                nc.vector.tensor_mul(sq[:], h1pre[:, kb, :], h1pre[:, kb, :])
                for c in range(2):
                    _mm(nc, musum[:, c, :], ones_col[:],
                        h1pre[:, kb, c * 512:(c + 1) * 512], kb == 0, kb == NKE - 1)
                    _mm(nc, sqsum[:, c, :], ones_col[:],
                        sq[:, c * 512:(c + 1) * 512], kb == 0, kb == NKE - 1)

            # preload the Sqrt act table off the LN1 critical path
            nc.scalar.activation(dumt[0:1, 1:2], epsP[0:1, :], AF.Sqrt)
            for i in range(NSTAGE):
                nc.sync.dma_start(w1stage[i][:], w1r_d[i])

            # depthwise conv (DVE) on the first two feature tiles
            for kb in range(2):
                pad = convp.tile([128, T + 2], bf16, tag="pad")
                nc.gpsimd.memset(pad[:, 0:1], 0.0)
                nc.gpsimd.memset(pad[:, T + 1:T + 2], 0.0)
                nc.vector.tensor_copy(pad[:, 1:T + 1], xt[:, kb, :])
                a1 = convp.tile([128, T], bf16, tag="a1")
                nc.vector.tensor_scalar_mul(a1[:], pad[:, 0:T], cwbc[:, 0:1])
                a2 = convp.tile([128, T], bf16, tag="a2")
                nc.vector.scalar_tensor_tensor(
                    a2[:], pad[:, 1:T + 1], cwbc[:, 1:2], a1[:], OP.mult, OP.add)
                a3 = convp.tile([128, T], bf16, tag="a3")
                nc.vector.scalar_tensor_tensor(
                    a3[:], pad[:, 2:T + 2], cwbc[:, 2:3], a2[:], OP.mult, OP.add)
                nc.vector.tensor_add(h1pre[:, kb, :], a3[:], xt[:, kb, :])
                ln1_k(kb)

            # attention out-projection with residual seeded via identity
            for m in range(8):
                wt = wop.tile([128, 8, 128], bf16, tag="wo")
                nc.sync.dma_start(wt[:], wor_d[m])
                for n in range(2):
                    ps = opsp.tile([128, 512], f32)
                    _mm(nc, ps[:], ident[:], xt[:, 2 + m, n * 512:(n + 1) * 512],
                        True, False)
                    for k in range(NKD):
                        _mm(nc, ps[:], wt[:, k, :],
                            attT[:, k, n * 512:(n + 1) * 512], False, k == NKD - 1)
                    nc.scalar.activation(
                        h1pre[:, 2 + m, n * 512:(n + 1) * 512], ps[:], AF.Identity,
                        bias=boP[:, m:m + 1])
                ln1_k(2 + m)

            # LayerNorm 1 statistics + normalize (feature axis = partitions).
            # Chain split across DVE/Act/Pool to shorten the critical path.
            mu = vecp.tile([1, T], f32, tag="mu")
            nc.vector.tensor_scalar_mul(
                mu[:], musum.rearrange("p a b -> p (a b)"), 1.0 / EMB)
            mu2 = vecp.tile([1, T], f32, tag="mu2s")
            nc.scalar.activation(mu2[:], mu[:], AF.Square)
            ex2 = vecp.tile([1, T], f32, tag="ex2")
            nc.gpsimd.tensor_scalar(
                ex2[:], sqsum.rearrange("p a b -> p (a b)"), 1.0 / EMB, EPS,
                OP.mult, OP.add)
            muB = vecp.tile([1, T], bf16, tag="muB")
            with nc.allow_low_precision(reason="bf16 LN1 factors"):
                nc.gpsimd.tensor_copy(muB[:], mu[:])
            muF = vecp.tile([128, T], bf16, tag="muF")
            nc.gpsimd.partition_broadcast(muF[:], muB[0:1, :])
            var = vecp.tile([1, T], f32, tag="var")
            nc.vector.tensor_sub(var[:], ex2[:], mu2[:])  # includes +eps
            vrec = vecp.tile([1, T], f32, tag="mu2s")
            nc.vector.reciprocal(vrec[:], var[:])
            rs = vecp.tile([1, T], f32, tag="ex2")
            nc.scalar.activation(rs[:], vrec[:], AF.Sqrt)
            rsB = vecp.tile([1, T], bf16, tag="rsB")
            with nc.allow_low_precision(reason="bf16 LN1 factors"):
                nc.vector.tensor_mul(rsB[:], rs[:], seq_row[:])  # fold seq_mask
            rsF = vecp.tile([128, T], bf16, tag="rsF")
            nc.gpsimd.partition_broadcast(rsF[:], rsB[0:1, :])
            for kb in range(NKE):
                t1 = sqp.tile([128, T], bf16, tag="t1")
                nc.vector.tensor_sub(t1[:], h1pre[:, kb, :], muF[:])
                t2 = sqp.tile([128, T], bf16, tag="t2")
                nc.vector.tensor_mul(t2[:], t1[:], rsF[:])
                nc.scalar.activation(
                    h1[:, kb, :], t2[:], AF.Identity,
                    bias=beta1P[:, kb:kb + 1], scale=g1P[:, kb:kb + 1])

        xtp.release()
        attp.release()
        h1prep.release()

        # ---------------- FFN1: ffb[m] = relu(h1 @ w1 + b1), all resident --
        outp = tc.alloc_tile_pool(name="outp", bufs=1)
        oacc = outp.tile([128, NKE, T], bf16)
        ffbp = tc.alloc_tile_pool(name="ffbp", bufs=1)
        ffb = ffbp.tile([128, NMF, T], bf16)
        w2ctx = tc.tile_pool(name="w2p", bufs=2)
        w2p = w2ctx.__enter__()
        w2ts = {}

        def load_w2(e):
            t = w2p.tile([128, 40, 128], bf16, tag="w2t")
            nc.sync.dma_start(t[:], w2r_d[e])
            w2ts[e] = t

        with (
            tc.tile_pool(name="w1p", bufs=3) as w1p,
            tc.tile_pool(name="ps1", bufs=3, space="PSUM") as ps1,
        ):
            for mf in range(NMF):
                if mf < NSTAGE:
                    w1t = w1stage[mf]
                else:
                    w1t = w1p.tile([128, 10, 128], bf16, tag="w1t")
                    nc.sync.dma_start(w1t[:], w1r_d[mf])
                if mf == 6:
                    load_w2(0)
                if mf == 24:
                    load_w2(1)
                ps = ps1.tile([128, 2, 512], f32)
                for k in range(NKE):
                    for c in range(2):
                        _mm(nc, ps[:, c, :], w1t[:, k, :],
                            h1[:, k, c * 512:(c + 1) * 512], k == 0, k == NKE - 1)
                nc.scalar.activation(
                    ffb[:, mf, :], ps.rearrange("p a b -> p (a b)"),
                    AF.Relu, bias=b1P[:, mf:mf + 1])

        # ---------------- FFN2 + LayerNorm 2 (feature-major) ---------------
        with (
            tc.tile_pool(name="ps2", bufs=2, space="PSUM") as ps2,
            tc.tile_pool(name="lnps2", bufs=1, space="PSUM") as lnps2,
            tc.tile_pool(name="sq2p", bufs=3) as sq2p,
            tc.tile_pool(name="vec2p", bufs=1) as vec2p,
        ):
            musum2 = lnps2.tile([1, 2, 512], f32, tag="musum2")
            sqsum2 = lnps2.tile([1, 2, 512], f32, tag="sqsum2")

            def emit_stats2(es, sqs):
                for c in range(2):
                    _mm(nc, musum2[:, c, :], ones_col[:],
                        oacc[:, es, c * 512:(c + 1) * 512], es == 0, es == NKE - 1)
                    _mm(nc, sqsum2[:, c, :], ones_col[:],
                        sqs[:, c * 512:(c + 1) * 512], es == 0, es == NKE - 1)

            pending_stats = None
            for e in range(NKE):
                if e + 1 < NKE and e + 1 not in w2ts:
                    load_w2(e + 1)
                w2t = w2ts.pop(e)
                pso = ps2.tile([128, 2, 512], f32)
                for k in range(NMF):
                    for c in range(2):
                        _mm(nc, pso[:, c, :], w2t[:, k, :],
                            ffb[:, k, c * 512:(c + 1) * 512], k == 0, k == NMF - 1)
                    if k == 8 and pending_stats is not None:
                        # stats for e-1 land mid-sweep so the PE never waits
                        # on the DVE epilogue of tile e-1
                        emit_stats2(*pending_stats)
                        pending_stats = None
                # oacc[e] = (pso + b2) + h1[e]   (residual2)
                nc.vector.scalar_tensor_tensor(
                    oacc[:, e, :], pso.rearrange("p a b -> p (a b)"),
                    b2P[:, e:e + 1], h1[:, e, :], OP.add, OP.add)
                sq = sq2p.tile([128, T], bf16, tag="sq2")
                nc.vector.tensor_mul(sq[:], oacc[:, e, :], oacc[:, e, :])
                pending_stats = (e, sq)
            emit_stats2(*pending_stats)

            mu = vec2p.tile([1, T], f32, tag="mu2")
            nc.vector.tensor_scalar_mul(
                mu[:], musum2.rearrange("p a b -> p (a b)"), 1.0 / EMB)
            mu2 = vec2p.tile([1, T], f32, tag="mu2s2")
            nc.scalar.activation(mu2[:], mu[:], AF.Square)
            ex2 = vec2p.tile([1, T], f32, tag="ex22")
            nc.gpsimd.tensor_scalar(
                ex2[:], sqsum2.rearrange("p a b -> p (a b)"), 1.0 / EMB, EPS,
                OP.mult, OP.add)
            muB = vec2p.tile([1, T], bf16, tag="muB2")
            with nc.allow_low_precision(reason="bf16 LN2 factors"):
                nc.gpsimd.tensor_copy(muB[:], mu[:])
            muF = vec2p.tile([128, T], bf16, tag="muF2")
            nc.gpsimd.partition_broadcast(muF[:], muB[0:1, :])
            var = vec2p.tile([1, T], f32, tag="var2")
            nc.vector.tensor_sub(var[:], ex2[:], mu2[:])  # includes +eps
            vrec = vec2p.tile([1, T], f32, tag="mu2s2")
            nc.vector.reciprocal(vrec[:], var[:])
            rs = vec2p.tile([1, T], f32, tag="ex22")
            nc.scalar.activation(rs[:], vrec[:], AF.Sqrt)
            rsB = vec2p.tile([1, T], bf16, tag="rsB2")
            with nc.allow_low_precision(reason="bf16 LN2 factors"):
                nc.vector.tensor_mul(rsB[:], rs[:], seq_row[:])
            rsF = vec2p.tile([128, T], bf16, tag="rsF2")
            nc.gpsimd.partition_broadcast(rsF[:], rsB[0:1, :])
            for e in range(NKE):
                t1 = sq2p.tile([128, T], bf16, tag="t12")
                nc.vector.tensor_sub(t1[:], oacc[:, e, :], muF[:])
                t2 = sq2p.tile([128, T], bf16, tag="t22")
                nc.vector.tensor_mul(t2[:], t1[:], rsF[:])
                nc.scalar.activation(
                    oacc[:, e, :], t2[:], AF.Identity,
                    bias=beta2P[:, e:e + 1], scale=g2P[:, e:e + 1])

        # ---------------- transpose to row-major + store --------------------
        w2ctx.__exit__(None, None, None)
        ffbp.release()
        with (
            tc.tile_pool(name="psT", bufs=3, space="PSUM") as psT,
            tc.tile_pool(name="obuf", bufs=3) as obuf,
        ):
            for tb in range(NT):
                pt = psT.tile([128, NKE, 128], bf16)
                for e in range(NKE):
                    nc.tensor.matmul(
                        pt[:, e, :], oacc[:, e, tb * 128:(tb + 1) * 128],
                        ident[:], start=True, stop=True, is_transpose=True)
                ob = obuf.tile([128, EMB], f32)
                if tb % 2 == 0:
                    nc.scalar.activation(
                        ob[:], pt.rearrange("p a b -> p (a b)"), AF.Identity)
                else:
                    nc.vector.tensor_copy(ob[:], pt.rearrange("p a b -> p (a b)"))
                nc.sync.dma_start(out_d[tb * 128:(tb + 1) * 128, :], ob[:])

        outp.release()
        h1p.release()
        constp.release()

    return nc


def _split_matmul_waits(bj: bytes) -> bytes:
    """Walrus codegen allows only one sync-wait on Matmult/DMACopy
    instructions; hoist extra waits onto a preceding EventSemaphore."""
    d = json.loads(bj)
    n = 0
    for f in d["functions"]:
        for blk in f["blocks"]:
            out = []
            for inst in blk["instructions"]:
                si = inst.get("sync_info")
                if (si and si.get("on_wait") and len(si["on_wait"]) >= 2
                        and inst.get("opcode") != "EventSemaphore"):
                    waits = si["on_wait"]
                    for w in waits[:-1]:
                        out.append({
                            "debug": inst.get("debug"),
                            "engine": inst["engine"],
                            "ins": [],
                            "outs": [],
                            "name": f"waitfix_{n}",
                            "opcode": "EventSemaphore",
                            "sync_info": {"on_update": [], "on_wait": [w]},
                        })
                        n += 1
                    si["on_wait"] = waits[-1:]
                out.append(inst)
            blk["instructions"] = out
    return json.dumps(d).encode()


_NC_CACHE = None


def _get_nc():
    global _NC_CACHE
    if _NC_CACHE is None:
        nc = build_nc()
        orig = nc.to_json_bytes
        nc.to_json_bytes = lambda: _split_matmul_waits(orig())
        _NC_CACHE = nc
    return _NC_CACHE


def _prep_core_inputs(x_b, mask_b, seq_b, conv_w, wq, bq, wk, bk, wv, bv, wo, bo,
                      w1, b1, w2, b2, g1, beta1, g2, beta2):
    f = np.float32
    bf = ml_dtypes.bfloat16
    mask_b = np.asarray(mask_b)
    masked = (mask_b != 0).astype(f)  # reference: att_mask != 0 -> -1e9 score
    return {
        "xt": np.ascontiguousarray(x_b.T).astype(bf),
        "wv": np.ascontiguousarray(wv).astype(bf),
        "wqr": np.ascontiguousarray(
            wq.reshape(8, 128, 8, 128).transpose(2, 1, 0, 3)).astype(bf),
        "wkr": np.ascontiguousarray(
            wk.reshape(8, 128, 8, 128).transpose(2, 1, 0, 3)).astype(bf),
        "wor": np.ascontiguousarray(
            wo.reshape(8, 128, 8, 128).transpose(2, 1, 0, 3)).astype(bf),
        "w1r": np.ascontiguousarray(
            w1.reshape(10, 128, 40, 128).transpose(2, 1, 0, 3)).astype(bf),
        "w2r": np.ascontiguousarray(
            w2.reshape(40, 128, 10, 128).transpose(2, 1, 0, 3)).astype(bf),
        "bvf": np.tile(np.asarray(bv, f)[None, :], (128, 1)).astype(bf),
        "mbias": np.ascontiguousarray(
            (MASK_NEG * masked).reshape(8, 128).T.astype(f)),
        "bqp": np.ascontiguousarray(np.asarray(bq, f).reshape(8, 128).T),
        "bkp": np.ascontiguousarray(np.asarray(bk, f).reshape(8, 128).T),
        "bop": np.ascontiguousarray(np.asarray(bo, f).reshape(8, 128).T),
        "b1p": np.ascontiguousarray(np.asarray(b1, f).reshape(40, 128).T),
        "b2p": np.ascontiguousarray(np.asarray(b2, f).reshape(10, 128).T),
        "g1p": np.ascontiguousarray(np.asarray(g1, f).reshape(10, 128).T),
        "beta1p": np.ascontiguousarray(np.asarray(beta1, f).reshape(10, 128).T),
        "g2p": np.ascontiguousarray(np.asarray(g2, f).reshape(10, 128).T),
        "beta2p": np.ascontiguousarray(np.asarray(beta2, f).reshape(10, 128).T),
        "cwbc": np.tile(np.asarray(conv_w, f).reshape(K)[None, :], (128, 1)),
        "seqrow": np.ascontiguousarray(np.asarray(seq_b, f).reshape(1, T)),
        "onescol": np.ones((128, 1), bf),
        "ident": np.eye(128, dtype=f).astype(bf),
    }


def kernel(x, att_mask, seq_mask, conv_w, wq, bq, wk, bk, wv, bv, wo, bo,
           w1, b1, w2, b2, g1, beta1, g2, beta2, _trace=False):
    from concourse.bass_utils import run_bass_kernel_spmd

    nc = _get_nc()
    x = np.asarray(x, dtype=np.float32)
    in_maps = []
    for b in range(B):
        in_maps.append(_prep_core_inputs(
            x[b], np.asarray(att_mask)[b], np.asarray(seq_mask)[b, :, 0],
            np.asarray(conv_w), np.asarray(wq), np.asarray(bq), np.asarray(wk),
            np.asarray(bk), np.asarray(wv), np.asarray(bv), np.asarray(wo),
            np.asarray(bo), np.asarray(w1), np.asarray(b1), np.asarray(w2),
            np.asarray(b2), np.asarray(g1), np.asarray(beta1), np.asarray(g2),
            np.asarray(beta2)))
    res = run_bass_kernel_spmd(nc, in_maps, list(range(B)), trace=_trace)
    out = np.stack([res.results[i]["out"] for i in range(B)], axis=0)
    if _trace:
        return out, res
    return out


# revision 24
# speedup vs baseline: 1.3328x; 1.0502x over previous
"""Trainium2 Bass/Tile kernel for nn_EncoderLayer (dense transformer block).

Strategy: pure data-parallel over batch (B=8 -> 1 batch element per core, no
collectives). Per core, activations are kept feature-major ([D, T]) in bf16
(same PE matmul rate as fp32r, half the DMA/SBUF, 2x DVE). Attention folds
the key mask into the exp bias (per-partition = per-key) and appends a ones
column to V so P@V yields the softmax normalizer for free; the attention
inner loop is software-pipelined (scores lag PV by one time-tile) so the
Activation engine's exp stream never starves. h1 stays resident in SBUF
(no DRAM round trip). FFN1 output (all 40 row-tiles) stays resident in bf16,
so FFN2 accumulates entirely in PSUM feature-major (no SBUF accumulation
adds) and LayerNorm2 statistics stream on the PE during the k-sweep; the
final transpose to row-major is done with PE transpose matmuls. Partition
broadcasts for LN run on the otherwise-idle GpSimd/Pool engine.
"""

import json
import sys

if "/opt/trn_rl_repo" not in sys.path:
    sys.path.insert(0, "/opt/trn_rl_repo")

import numpy as np
import ml_dtypes

import concourse.bass as bass
import concourse.mybir as mybir
import concourse.tile as tile
from concourse import library_config

B, T, CC, DM, H, DH, DFF, K = 8, 1024, 256, 1024, 16, 64, 5120, 3
EMB = CC + DM  # 1280
EPS = 1e-6
f32 = mybir.dt.float32
bf16 = mybir.dt.bfloat16
AF = mybir.ActivationFunctionType
OP = mybir.AluOpType

NT = T // 128          # 8 time tiles
NKE = EMB // 128       # 10 embed k-tiles
NKD = DM // 128        # 8 d_model k-tiles
NMF = DFF // 128       # 40 d_ff tiles
HV = DH + 1            # 65: per-head V columns + normalizer ones column
MASK_NEG = -60000.0    # exp(-60000 + s/8) == 0.0 in f32


def _mm(nc, out, lhsT, rhs, start, stop):
    nc.tensor.matmul(out, lhsT, rhs, start=start, stop=stop)


def _ln_factors(nc, tc, pool, pspool, stat, seqP, ident, epsP, sfx):
    """From row-major PSUM sums stat[:, 0:8]=sum(x), stat[:, 8:16]=sum(x^2)
    (per (t%128, t//128)), produce muF/rsF [128, T] bf16 broadcast tiles.
    All elementwise work is on [128, 8] tiles; rows are built with 1-column
    PE transposes."""
    f32 = mybir.dt.float32
    bf16 = mybir.dt.bfloat16
    AF = mybir.ActivationFunctionType
    OP = mybir.AluOpType
    mur = pool.tile([128, 8], f32, tag="mur" + sfx)
    nc.vector.tensor_scalar_mul(mur[:], stat[:, 0:8], 1.0 / EMB)
    mu2r = pool.tile([128, 8], f32, tag="mu2r" + sfx)
    nc.scalar.activation(mu2r[:], mur[:], AF.Square)
    ex2r = pool.tile([128, 8], f32, tag="ex2r" + sfx)
    nc.vector.tensor_scalar(stat[:, 8:16], 1.0 / EMB, EPS, OP.mult, OP.add,
                            out=ex2r[:]) if False else         nc.vector.tensor_scalar(ex2r[:], stat[:, 8:16], 1.0 / EMB, EPS,
                                OP.mult, OP.add)
    varr = pool.tile([128, 8], f32, tag="varr" + sfx)
    nc.vector.tensor_sub(varr[:], ex2r[:], mu2r[:])  # includes +eps
    vrecr = pool.tile([128, 8], f32, tag="mu2r" + sfx)
    nc.vector.reciprocal(vrecr[:], varr[:])
    rsr = pool.tile([128, 8], f32, tag="ex2r" + sfx)
    nc.scalar.activation(rsr[:], vrecr[:], AF.Sqrt)
    muB = pool.tile([128, 8], bf16, tag="muB" + sfx)
    with nc.allow_low_precision(reason="bf16 LN factors"):
        nc.vector.tensor_copy(muB[:], mur[:])
    rsB = pool.tile([128, 8], bf16, tag="rsB" + sfx)
    with nc.allow_low_precision(reason="bf16 LN factors"):
        nc.vector.tensor_mul(rsB[:], rsr[:], seqP[:])  # fold seq_mask
    rows = pspool.tile([1, 2, T], bf16, tag="lnrows" + sfx)
    for tc_ in range(NT):
        nc.tensor.matmul(rows[:, 0, tc_ * 128:(tc_ + 1) * 128],
                         muB[:, tc_:tc_ + 1], ident[:],
                         start=True, stop=True, is_transpose=True)
        nc.tensor.matmul(rows[:, 1, tc_ * 128:(tc_ + 1) * 128],
                         rsB[:, tc_:tc_ + 1], ident[:],
                         start=True, stop=True, is_transpose=True)
    muF = pool.tile([128, T], bf16, tag="muF" + sfx)
    nc.gpsimd.partition_broadcast(muF[:], rows[0:1, 0, :])
    rsF = pool.tile([128, T], bf16, tag="rsF" + sfx)
    nc.gpsimd.partition_broadcast(rsF[:], rows[0:1, 1, :])
    return muF, rsF


def build_nc():
    nc = bass.Bass()

    xt_d = nc.declare_dram_parameter("xt", [EMB, T], bf16, isOutput=False)
    wv_d = nc.declare_dram_parameter("wv", [DM, DM], bf16, isOutput=False)
    wqr_d = nc.declare_dram_parameter("wqr", [8, 128, 8, 128], bf16, isOutput=False)
    wkr_d = nc.declare_dram_parameter("wkr", [8, 128, 8, 128], bf16, isOutput=False)
    wor_d = nc.declare_dram_parameter("wor", [8, 128, 8, 128], bf16, isOutput=False)
    w1r_d = nc.declare_dram_parameter("w1r", [40, 128, 10, 128], bf16, isOutput=False)
    w2r_d = nc.declare_dram_parameter("w2r", [10, 128, 40, 128], bf16, isOutput=False)
    bvf_d = nc.declare_dram_parameter("bvf", [128, DM], bf16, isOutput=False)
    mbias_d = nc.declare_dram_parameter("mbias", [128, 8], f32, isOutput=False)
    bqp_d = nc.declare_dram_parameter("bqp", [128, 8], f32, isOutput=False)
    bkp_d = nc.declare_dram_parameter("bkp", [128, 8], f32, isOutput=False)
    bop_d = nc.declare_dram_parameter("bop", [128, 8], f32, isOutput=False)
    b1p_d = nc.declare_dram_parameter("b1p", [128, 40], f32, isOutput=False)
    b2p_d = nc.declare_dram_parameter("b2p", [128, 10], f32, isOutput=False)
    g1p_d = nc.declare_dram_parameter("g1p", [128, 10], f32, isOutput=False)
    beta1p_d = nc.declare_dram_parameter("beta1p", [128, 10], f32, isOutput=False)
    g2p_d = nc.declare_dram_parameter("g2p", [128, 10], f32, isOutput=False)
    beta2p_d = nc.declare_dram_parameter("beta2p", [128, 10], f32, isOutput=False)
    cwbc_d = nc.declare_dram_parameter("cwbc", [128, K], f32, isOutput=False)
    seqp_d = nc.declare_dram_parameter("seqp", [128, 8], f32, isOutput=False)
    onescol_d = nc.declare_dram_parameter("onescol", [128, 1], bf16, isOutput=False)
    ident_d = nc.declare_dram_parameter("ident", [128, 128], bf16, isOutput=False)
    out_d = nc.declare_dram_parameter("out", [T, EMB], f32, isOutput=True)

    with tile.TileContext(nc) as tc:
        nc.gpsimd.load_library(library_config.proxy)

        # ---------------- persistent pools (alloc in reverse-death order) ---
        constp = tc.alloc_tile_pool(name="constp", bufs=1)
        h1p = tc.alloc_tile_pool(name="h1p", bufs=1)
        h1 = h1p.tile([128, NKE, T], bf16)
        h1prep = tc.alloc_tile_pool(name="h1prep", bufs=1)
        h1pre = h1prep.tile([128, NKE, T], bf16)
        attp = tc.alloc_tile_pool(name="attp", bufs=1)
        attT = attp.tile([128, NKD, T], bf16)
        xtp = tc.alloc_tile_pool(name="xtp", bufs=1)
        xt = xtp.tile([128, NKE, T], bf16)

        # const tiles allocated now; DMAs deferred past the startup stream
        bvF = constp.tile([128, DM], bf16)
        mbias = constp.tile([128, 8], f32)
        bqP = constp.tile([128, 8], f32)
        bkP = constp.tile([128, 8], f32)
        boP = constp.tile([128, 8], f32)
        b1P = constp.tile([128, 40], f32)
        b2P = constp.tile([128, 10], f32)
        g1P = constp.tile([128, 10], f32)
        beta1P = constp.tile([128, 10], f32)
        g2P = constp.tile([128, 10], f32)
        beta2P = constp.tile([128, 10], f32)
        cwbc = constp.tile([128, K], f32)
        seqP = constp.tile([128, 8], f32)
        ones_col = constp.tile([128, 1], bf16)
        ident = constp.tile([128, 128], bf16)
        epsP = constp.tile([128, 1], f32)
        nc.gpsimd.memset(epsP[:], EPS)
        dumt = constp.tile([1, 2], f32)
        NSTAGE = 6
        w1stage = [constp.tile([128, 10, 128], bf16, name=f"w1s{i}")
                   for i in range(NSTAGE)]

        def emit_const_dmas():
            nc.sync.dma_start(mbias[:], mbias_d[:])
            nc.sync.dma_start(bqP[:], bqp_d[:])
            nc.sync.dma_start(bkP[:], bkp_d[:])
            nc.sync.dma_start(boP[:], bop_d[:])
            nc.sync.dma_start(b1P[:], b1p_d[:])
            nc.sync.dma_start(b2P[:], b2p_d[:])
            nc.sync.dma_start(g1P[:], g1p_d[:])
            nc.sync.dma_start(beta1P[:], beta1p_d[:])
            nc.sync.dma_start(g2P[:], g2p_d[:])
            nc.sync.dma_start(beta2P[:], beta2p_d[:])
            nc.sync.dma_start(cwbc[:], cwbc_d[:])
            nc.sync.dma_start(seqP[:], seqp_d[:])
            nc.sync.dma_start(ones_col[:], onescol_d[:])
            nc.sync.dma_start(ident[:], ident_d[:])
            for kk in range(2):  # conv feature tiles
                nc.sync.dma_start(xt[:, kk, :], xt_d[kk * 128:(kk + 1) * 128, :])

        vp = tc.alloc_tile_pool(name="vp", bufs=1)
        vaug = vp.tile([128, NT, H * HV], bf16)
        # normalizer ones column (col DH of each head slot)
        ocols = vaug.rearrange("p j (h c) -> p (j h) c", c=HV)[:, :, DH:HV]
        nc.gpsimd.memset(ocols, 1.0)

        qkp = tc.alloc_tile_pool(name="qkp", bufs=1)
        qt = qkp.tile([128, NKD, T], bf16)
        kt = qkp.tile([128, NKD, T], bf16)

        with tc.tile_pool(name="wqp", bufs=3) as wqp:
            qk_wts = []

            def emit_qk_load(i):
                wdram = wqr_d if i < 8 else wkr_d
                wt = wqp.tile([128, 8, 128], bf16, tag="wt")
                nc.sync.dma_start(wt[:], wdram[i % 8])
                qk_wts.append(wt)

            # ------------ V projection (row-major, bias, augmented) --------
            with (
                tc.tile_pool(name="wvp", bufs=3) as wvp,
                tc.tile_pool(name="vps", bufs=8, space="PSUM") as vps,
            ):
                for n in range(2):
                    pss = [vps.tile([128, 512], f32, name=f"vps{i}", tag="vps")
                           for i in range(NT)]
                    for k in range(NKD):
                        if n == 0:  # startup: interleave x and wv streams
                            nc.sync.dma_start(
                                xt[:, 2 + k, :],
                                xt_d[(2 + k) * 128:(3 + k) * 128, :])
                        if n == 1 and k == 2:
                            emit_const_dmas()
                        if n == 1 and k == 5:
                            emit_qk_load(0)
                            emit_qk_load(1)
                        wvt = wvp.tile([128, 512], bf16)
                        nc.sync.dma_start(
                            wvt[:],
                            wv_d[k * 128:(k + 1) * 128, n * 512:(n + 1) * 512])
                        for i in range(NT):
                            _mm(nc, pss[i][:], xt[:, 2 + k, i * 128:(i + 1) * 128],
                                wvt[:], k == 0, k == NKD - 1)
                    if n == 0:
                        nc.sync.dma_start(bvF[:], bvf_d[:])
                        # preload the Exp activation table before attention
                        nc.scalar.activation(dumt[0:1, 0:1], epsP[0:1, :], AF.Exp)
                    for i in range(NT):
                        dest = vaug[:, i, :].rearrange("p (h c) -> p h c", c=HV)
                        dest = dest[:, n * 8:(n + 1) * 8, 0:DH]
                        eng = nc.vector if i % 8 < 5 else nc.gpsimd
                        eng.tensor_add(dest, pss[i][:],
                                       bvF[:, n * 512:(n + 1) * 512])

            # ------------ Q/K projections (feature-major) ------------------
            with tc.tile_pool(name="qps", bufs=2, space="PSUM") as qps:
                for i in range(16):
                    if i + 2 < 16:
                        emit_qk_load(i + 2)
                    m = i % 8
                    dst, biasP = (qt, bqP) if i < 8 else (kt, bkP)
                    wt = qk_wts[i]
                    ps = qps.tile([128, 2, 512], f32)
                    for n in range(2):
                        for k in range(NKD):
                            _mm(nc, ps[:, n, :], wt[:, k, :],
                                xt[:, 2 + k, n * 512:(n + 1) * 512],
                                k == 0, k == NKD - 1)
                    nc.vector.tensor_scalar_add(
                        dst[:, m, :], ps.rearrange("p a b -> p (a b)"),
                        biasP[:, m:m + 1])

        # ---------------- attention (single head, lag-1 pipelined) ---------
        with (
            tc.tile_pool(name="upool", bufs=3) as upool,
            tc.tile_pool(name="normp", bufs=2) as normp,
            tc.tile_pool(name="bcp", bufs=2) as bcp,
            tc.tile_pool(name="sps", bufs=2, space="PSUM") as spsp,
            tc.tile_pool(name="aps", bufs=2, space="PSUM") as apsp,
        ):
            def scores_exp(h, jt):
                prow = (h % 2) * 64
                ktile = h // 2
                sps = spsp.tile([128, 2, 512], f32, name="sps", tag="sps")
                klhs = kt[prow:prow + 64, ktile, jt * 128:(jt + 1) * 128]
                for c in range(2):
                    _mm(nc, sps[:, c, :], klhs,
                        qt[prow:prow + 64, ktile, c * 512:(c + 1) * 512],
                        True, True)
                u = upool.tile([128, T], bf16, name="u", tag="u")
                nc.scalar.activation(
                    u[:], sps.rearrange("p a b -> p (a b)"), AF.Exp,
                    scale=0.125, bias=mbias[:, jt:jt + 1])
                return u

            def pv(h, jt, u, aps):
                vlhs = vaug[:, jt, h * HV:(h + 1) * HV]
                for c in range(2):
                    _mm(nc, aps[:, c, :], vlhs,
                        u[:, c * 512:(c + 1) * 512], jt == 0, jt == NT - 1)

            def evacuate(h, aps):
                # 1/normalizer; unnormalized attention rows -> attT
                prow = (h % 2) * 64
                ktile = h // 2
                nt_ = normp.tile([1, T], bf16, name="nt", tag="nt")
                with nc.allow_low_precision(reason="bf16 softmax normalizer"):
                    nc.vector.reciprocal(
                        nt_[:], aps[DH:HV, :, :].rearrange("p a b -> p (a b)"))
                nc.vector.tensor_copy(
                    attT[prow:prow + 64, ktile, :],
                    aps[0:DH, :, :].rearrange("p a b -> p (a b)"))
                return nt_

            def finalize(h, nt_):
                prow = (h % 2) * 64
                ktile = h // 2
                bc = bcp.tile([64, T], bf16, name="bc", tag="bc")
                nc.gpsimd.partition_broadcast(bc[:], nt_[0:1, :])
                nc.vector.tensor_mul(
                    attT[prow:prow + 64, ktile, :],
                    attT[prow:prow + 64, ktile, :], bc[:])

            pending = None  # (h, norm_tile) awaiting broadcast+scale
            for h in range(H):
                aps = apsp.tile([HV, 2, 512], f32, name="aps", tag="aps")
                us = [scores_exp(h, 0), scores_exp(h, 1)]
                for jt in range(NT):
                    if jt + 2 < NT:
                        us.append(scores_exp(h, jt + 2))
                    pv(h, jt, us[jt], aps)
                nt_ = evacuate(h, aps)
                if pending is not None:
                    finalize(*pending)
                pending = (h, nt_)
            finalize(*pending)

        qkp.release()
        vp.release()

        # ---------------- h1pre = concat(conv, att@wo + bo) + x ------------
        with (
            tc.tile_pool(name="convp", bufs=2) as convp,
            tc.tile_pool(name="wop", bufs=3) as wop,
            tc.tile_pool(name="ops", bufs=4, space="PSUM") as opsp,
            tc.tile_pool(name="lnps", bufs=1, space="PSUM") as lnps,
            tc.tile_pool(name="sqp", bufs=3) as sqp,
            tc.tile_pool(name="vecp", bufs=1) as vecp,
        ):
            stat = lnps.tile([128, 16], f32, tag="stat")

            def ln1_k(kb):
                sq = sqp.tile([128, T], bf16, tag="sq")
                nc.vector.tensor_mul(sq[:], h1pre[:, kb, :], h1pre[:, kb, :])
                for tc in range(NT):
                    nc.tensor.matmul(
                        stat[:, tc:tc + 1],
                        h1pre[:, kb, tc * 128:(tc + 1) * 128], ones_col[:],
                        start=kb == 0 and tc == 0,
                        stop=kb == NKE - 1 and tc == NT - 1,
                        skip_group_check=True)
                    nc.tensor.matmul(
                        stat[:, 8 + tc:9 + tc],
                        sq[:, tc * 128:(tc + 1) * 128], ones_col[:],
                        start=False, stop=False, skip_group_check=True)

            # preload the Sqrt act table off the LN1 critical path
            nc.scalar.activation(dumt[0:1, 1:2], epsP[0:1, :], AF.Sqrt)
            for i in range(NSTAGE):
                nc.sync.dma_start(w1stage[i][:], w1r_d[i])

            # depthwise conv (DVE) on the first two feature tiles
            for kb in range(2):
                pad = convp.tile([128, T + 2], bf16, tag="pad")
                nc.gpsimd.memset(pad[:, 0:1], 0.0)
                nc.gpsimd.memset(pad[:, T + 1:T + 2], 0.0)
                nc.vector.tensor_copy(pad[:, 1:T + 1], xt[:, kb, :])
                a1 = convp.tile([128, T], bf16, tag="a1")
                nc.vector.tensor_scalar_mul(a1[:], pad[:, 0:T], cwbc[:, 0:1])
                a2 = convp.tile([128, T], bf16, tag="a2")
                nc.vector.scalar_tensor_tensor(
                    a2[:], pad[:, 1:T + 1], cwbc[:, 1:2], a1[:], OP.mult, OP.add)
                a3 = convp.tile([128, T], bf16, tag="a3")
                nc.vector.scalar_tensor_tensor(
                    a3[:], pad[:, 2:T + 2], cwbc[:, 2:3], a2[:], OP.mult, OP.add)
                nc.vector.tensor_add(h1pre[:, kb, :], a3[:], xt[:, kb, :])
                ln1_k(kb)

            # attention out-projection with residual seeded via identity
            for m in range(8):
                wt = wop.tile([128, 8, 128], bf16, tag="wo")
                nc.sync.dma_start(wt[:], wor_d[m])
                for n in range(2):
                    ps = opsp.tile([128, 512], f32)
                    _mm(nc, ps[:], ident[:], xt[:, 2 + m, n * 512:(n + 1) * 512],
                        True, False)
                    for k in range(NKD):
                        _mm(nc, ps[:], wt[:, k, :],
                            attT[:, k, n * 512:(n + 1) * 512], False, k == NKD - 1)
                    nc.scalar.activation(
                        h1pre[:, 2 + m, n * 512:(n + 1) * 512], ps[:], AF.Identity,
                        bias=boP[:, m:m + 1])
                ln1_k(2 + m)

            # LayerNorm 1 statistics: tiny [128, 8] row-major chain, then
            # PE transposes to a [1, T] row and Pool partition-broadcasts.
            muF, rsF = _ln_factors(
                nc, tc, vecp, lnps, stat, seqP, ident, epsP, "1")
            for kb in range(NKE):
                t1 = sqp.tile([128, T], bf16, tag="t1")
                nc.vector.tensor_sub(t1[:], h1pre[:, kb, :], muF[:])
                t2 = sqp.tile([128, T], bf16, tag="t2")
                nc.vector.tensor_mul(t2[:], t1[:], rsF[:])
                nc.scalar.activation(
                    h1[:, kb, :], t2[:], AF.Identity,
                    bias=beta1P[:, kb:kb + 1], scale=g1P[:, kb:kb + 1])

        xtp.release()
        attp.release()
        h1prep.release()

        # ---------------- FFN1: ffb[m] = relu(h1 @ w1 + b1), all resident --
        outp = tc.alloc_tile_pool(name="outp", bufs=1)
        oacc = outp.tile([128, NKE, T], bf16)
        ffbp = tc.alloc_tile_pool(name="ffbp", bufs=1)
        ffb = ffbp.tile([128, NMF, T], bf16)
        w2ctx = tc.tile_pool(name="w2p", bufs=2)
        w2p = w2ctx.__enter__()
        w2ts = {}

        def load_w2(e):
            t = w2p.tile([128, 40, 128], bf16, tag="w2t")
            nc.sync.dma_start(t[:], w2r_d[e])
            w2ts[e] = t

        with (
            tc.tile_pool(name="w1p", bufs=3) as w1p,
            tc.tile_pool(name="ps1", bufs=3, space="PSUM") as ps1,
        ):
            for mf in range(NMF):
                if mf < NSTAGE:
                    w1t = w1stage[mf]
                else:
                    w1t = w1p.tile([128, 10, 128], bf16, tag="w1t")
                    nc.sync.dma_start(w1t[:], w1r_d[mf])
                if mf == 6:
                    load_w2(0)
                if mf == 24:
                    load_w2(1)
                ps = ps1.tile([128, 2, 512], f32)
                for k in range(NKE):
                    for c in range(2):
                        _mm(nc, ps[:, c, :], w1t[:, k, :],
                            h1[:, k, c * 512:(c + 1) * 512], k == 0, k == NKE - 1)
                nc.scalar.activation(
                    ffb[:, mf, :], ps.rearrange("p a b -> p (a b)"),
                    AF.Relu, bias=b1P[:, mf:mf + 1])

        # ---------------- FFN2 + LayerNorm 2 (feature-major) ---------------
        with (
            tc.tile_pool(name="ps2", bufs=2, space="PSUM") as ps2,
            tc.tile_pool(name="lnps2", bufs=1, space="PSUM") as lnps2,
            tc.tile_pool(name="sq2p", bufs=3) as sq2p,
            tc.tile_pool(name="vec2p", bufs=1) as vec2p,
        ):
            stat2 = lnps2.tile([128, 16], f32, tag="stat2")

            def emit_stats2(es, sqs):
                for tc in range(NT):
                    nc.tensor.matmul(
                        stat2[:, tc:tc + 1],
                        oacc[:, es, tc * 128:(tc + 1) * 128], ones_col[:],
                        start=es == 0 and tc == 0,
                        stop=es == NKE - 1 and tc == NT - 1,
                        skip_group_check=True)
                    nc.tensor.matmul(
                        stat2[:, 8 + tc:9 + tc],
                        sqs[:, tc * 128:(tc + 1) * 128], ones_col[:],
                        start=False, stop=False, skip_group_check=True)

            pending_stats = None
            for e in range(NKE):
                if e + 1 < NKE and e + 1 not in w2ts:
                    load_w2(e + 1)
                w2t = w2ts.pop(e)
                pso = ps2.tile([128, 2, 512], f32)
                for k in range(NMF):
                    for c in range(2):
                        _mm(nc, pso[:, c, :], w2t[:, k, :],
                            ffb[:, k, c * 512:(c + 1) * 512], k == 0, k == NMF - 1)
                    if k == 8 and pending_stats is not None:
                        # stats for e-1 land mid-sweep so the PE never waits
                        # on the DVE epilogue of tile e-1
                        emit_stats2(*pending_stats)
                        pending_stats = None
                # oacc[e] = (pso + b2) + h1[e]   (residual2)
                nc.vector.scalar_tensor_tensor(
                    oacc[:, e, :], pso.rearrange("p a b -> p (a b)"),
                    b2P[:, e:e + 1], h1[:, e, :], OP.add, OP.add)
                sq = sq2p.tile([128, T], bf16, tag="sq2")
                nc.vector.tensor_mul(sq[:], oacc[:, e, :], oacc[:, e, :])
                pending_stats = (e, sq)
            emit_stats2(*pending_stats)

            muF, rsF = _ln_factors(
                nc, tc, vec2p, lnps2, stat2, seqP, ident, epsP, "2")
            for e in range(NKE):
                t1 = sq2p.tile([128, T], bf16, tag="t12")
                nc.vector.tensor_sub(t1[:], oacc[:, e, :], muF[:])
                t2 = sq2p.tile([128, T], bf16, tag="t22")
                nc.vector.tensor_mul(t2[:], t1[:], rsF[:])
                nc.scalar.activation(
                    oacc[:, e, :], t2[:], AF.Identity,
                    bias=beta2P[:, e:e + 1], scale=g2P[:, e:e + 1])

        # ---------------- transpose to row-major + store --------------------
        w2ctx.__exit__(None, None, None)
        ffbp.release()
        with (
            tc.tile_pool(name="psT", bufs=3, space="PSUM") as psT,
            tc.tile_pool(name="obuf", bufs=3) as obuf,
        ):
            for tb in range(NT):
                pt = psT.tile([128, NKE, 128], bf16)
                for e in range(NKE):
                    nc.tensor.matmul(
                        pt[:, e, :], oacc[:, e, tb * 128:(tb + 1) * 128],
                        ident[:], start=True, stop=True, is_transpose=True)
                ob = obuf.tile([128, EMB], f32)
                if tb % 2 == 0:
                    nc.scalar.activation(
                        ob[:], pt.rearrange("p a b -> p (a b)"), AF.Identity)
                else:
                    nc.vector.tensor_copy(ob[:], pt.rearrange("p a b -> p (a b)"))
                nc.sync.dma_start(out_d[tb * 128:(tb + 1) * 128, :], ob[:])

        outp.release()
        h1p.release()
        constp.release()

    return nc


def _split_matmul_waits(bj: bytes) -> bytes:
    """Walrus codegen allows only one sync-wait on Matmult/DMACopy
    instructions; hoist extra waits onto a preceding EventSemaphore."""
    d = json.loads(bj)
    n = 0
    for f in d["functions"]:
        for blk in f["blocks"]:
            out = []
            for inst in blk["instructions"]:
                si = inst.get("sync_info")
                if (si and si.get("on_wait") and len(si["on_wait"]) >= 2
                        and inst.get("opcode") != "EventSemaphore"):
                    waits = si["on_wait"]
                    for w in waits[:-1]:
                        out.append({
                            "debug": inst.get("debug"),
                            "engine": inst["engine"],
                            "ins": [],
                            "outs": [],
                            "name": f"waitfix_{n}",
                            "opcode": "EventSemaphore",
                            "sync_info": {"on_update": [], "on_wait": [w]},
                        })
                        n += 1
                    si["on_wait"] = waits[-1:]
                out.append(inst)
            blk["instructions"] = out
    return json.dumps(d).encode()


_NC_CACHE = None


def _get_nc():
    global _NC_CACHE
    if _NC_CACHE is None:
        nc = build_nc()
        orig = nc.to_json_bytes
        nc.to_json_bytes = lambda: _split_matmul_waits(orig())
        _NC_CACHE = nc
    return _NC_CACHE


def _prep_core_inputs(x_b, mask_b, seq_b, conv_w, wq, bq, wk, bk, wv, bv, wo, bo,
                      w1, b1, w2, b2, g1, beta1, g2, beta2):
    f = np.float32
    bf = ml_dtypes.bfloat16
    mask_b = np.asarray(mask_b)
    masked = (mask_b != 0).astype(f)  # reference: att_mask != 0 -> -1e9 score
    return {
        "xt": np.ascontiguousarray(x_b.T).astype(bf),
        "wv": np.ascontiguousarray(wv).astype(bf),
        "wqr": np.ascontiguousarray(
            wq.reshape(8, 128, 8, 128).transpose(2, 1, 0, 3)).astype(bf),
        "wkr": np.ascontiguousarray(
            wk.reshape(8, 128, 8, 128).transpose(2, 1, 0, 3)).astype(bf),
        "wor": np.ascontiguousarray(
            wo.reshape(8, 128, 8, 128).transpose(2, 1, 0, 3)).astype(bf),
        "w1r": np.ascontiguousarray(
            w1.reshape(10, 128, 40, 128).transpose(2, 1, 0, 3)).astype(bf),
        "w2r": np.ascontiguousarray(
            w2.reshape(40, 128, 10, 128).transpose(2, 1, 0, 3)).astype(bf),
        "bvf": np.tile(np.asarray(bv, f)[None, :], (128, 1)).astype(bf),
        "mbias": np.ascontiguousarray(
            (MASK_NEG * masked).reshape(8, 128).T.astype(f)),
        "bqp": np.ascontiguousarray(np.asarray(bq, f).reshape(8, 128).T),
        "bkp": np.ascontiguousarray(np.asarray(bk, f).reshape(8, 128).T),
        "bop": np.ascontiguousarray(np.asarray(bo, f).reshape(8, 128).T),
        "b1p": np.ascontiguousarray(np.asarray(b1, f).reshape(40, 128).T),
        "b2p": np.ascontiguousarray(np.asarray(b2, f).reshape(10, 128).T),
        "g1p": np.ascontiguousarray(np.asarray(g1, f).reshape(10, 128).T),
        "beta1p": np.ascontiguousarray(np.asarray(beta1, f).reshape(10, 128).T),
        "g2p": np.ascontiguousarray(np.asarray(g2, f).reshape(10, 128).T),
        "beta2p": np.ascontiguousarray(np.asarray(beta2, f).reshape(10, 128).T),
        "cwbc": np.tile(np.asarray(conv_w, f).reshape(K)[None, :], (128, 1)),
        "seqp": np.ascontiguousarray(np.asarray(seq_b, f).reshape(8, 128).T),
        "onescol": np.ones((128, 1), bf),
        "ident": np.eye(128, dtype=f).astype(bf),
    }


def kernel(x, att_mask, seq_mask, conv_w, wq, bq, wk, bk, wv, bv, wo, bo,
           w1, b1, w2, b2, g1, beta1, g2, beta2, _trace=False):
    from concourse.bass_utils import run_bass_kernel_spmd

    nc = _get_nc()
    x = np.asarray(x, dtype=np.float32)
    in_maps = []
    for b in range(B):
        in_maps.append(_prep_core_inputs(
            x[b], np.asarray(att_mask)[b], np.asarray(seq_mask)[b, :, 0],
            np.asarray(conv_w), np.asarray(wq), np.asarray(bq), np.asarray(wk),
            np.asarray(bk), np.asarray(wv), np.asarray(bv), np.asarray(wo),
            np.asarray(bo), np.asarray(w1), np.asarray(b1), np.asarray(w2),
            np.asarray(b2), np.asarray(g1), np.asarray(beta1), np.asarray(g2),
            np.asarray(beta2)))
    res = run_bass_kernel_spmd(nc, in_maps, list(range(B)), trace=_trace)
    out = np.stack([res.results[i]["out"] for i in range(B)], axis=0)
    if _trace:
        return out, res
    return out
